# revision 18
# baseline (speedup 1.0000x reference)
"""Trainium2 Bass kernel for a 4-step differentiable recurrent net forward pass.

Reference computation (B=8192, NI=512, NH=2048, NO=512, 4 steps):
    activs = 0; outputs = 0
    repeat 4x:  pre = hr * (x @ Wih.T + activs @ Whh.T + outputs @ Woh.T) + hb
                activs = per_neuron_act(pre)        # tanh/sigmoid/relu by i%3
    out = sigmoid(or * (x @ Wio.T + outputs @ Woo.T + activs @ Who.T) + ob)

`outputs` is never written inside the loop, so the Woh/Woo terms vanish and
the x-projection P = hr*(x@Wih.T)+hb is loop-invariant (computed once).

Strategy: data-parallel on batch across 8 cores (1024 rows each). On-core
everything is feature-major (features on SBUF partitions, batch on the free
axis), so each matmul is W_tile.T @ X^T with stationary weights. All matmuls
run in fp8 e4m3 with DoubleRow perf mode (two k-tiles per instruction, 2x
MAC throughput). Weights are scaled by S=256 host-side so their ~0.02-scale
values sit in e4m3's normal range; the 1/S is folded into the activation
instruction's input scale. Activations are quantized to e4m3 unscaled (they
are O(1)). PSUM accumulates in fp32 throughout, so only operand quantization
loses precision (~1.3e-2 rel err on the final sigmoid outputs).
Host-side prep: hidden neurons are permuted so the three activation groups
are contiguous, hr/or are folded into the weight matrices, weights are
packed so each loads as one large contiguous DMA, and hb/ob are applied as
per-partition bias APs.
"""

import os

import numpy as np
import ml_dtypes

import concourse.bass as bass
import concourse.tile as tile
from concourse import bacc, mybir
from concourse.bass_utils import run_bass_kernel_spmd

B, NI, NH, NO = 8192, 512, 2048, 512
N_STEPS = 4
N_CORES = 8
BL = B // N_CORES          # batch rows per core
CH = 512                   # batch chunk (one PSUM bank of fp32)
NCH = BL // CH             # 2 chunks per core
KI = NI // 128             # 4 k-tiles over inputs
KH = NH // 128             # 16 k/m-tiles over hidden
KO = NO // 128             # 4 m-tiles over outputs

FP8 = mybir.dt.float8e4
BF16 = mybir.dt.bfloat16
F32 = mybir.dt.float32
AF = mybir.ActivationFunctionType
DR = mybir.MatmulPerfMode.DoubleRow
E4 = ml_dtypes.float8_e4m3

WS = 256.0                 # weight scale into fp8 range
IWS = 1.0 / WS             # folded back out at activation time

# hidden neurons regrouped as [all tanh | all sigmoid | all relu]
_idx = np.arange(NH)
PERM = np.concatenate([_idx[_idx % 3 == 0], _idx[_idx % 3 == 1], _idx[_idx % 3 == 2]])
_B1 = int((_idx % 3 == 0).sum())           # 683
_B2 = _B1 + int((_idx % 3 == 1).sum())     # 1366

# per m-tile: the single activation function, or None for the two mixed tiles
_TILE_FUNC = []
for _m in range(KH):
    _lo, _hi = _m * 128, (_m + 1) * 128
    _fs = set()
    for _f, _a, _b in ((AF.Tanh, 0, _B1), (AF.Sigmoid, _B1, _B2), (AF.Relu, _B2, NH)):
        if max(_lo, _a) < min(_hi, _b):
            _fs.add(_f)
    _TILE_FUNC.append(_fs.pop() if len(_fs) == 1 else None)

# mixed tiles: (major_func applied everywhere, minor_func, mask column block)
# partition sub-ranges must be 32-aligned on TRN2, so the minority strip is
# fixed up with a full-tile ACT + copy_predicated against a {0,1} mask
_BOUNDARY = {
    _B1 // 128: (AF.Sigmoid, AF.Tanh, 0),    # tile 5: parts < 43 are tanh
    _B2 // 128: (AF.Sigmoid, AF.Relu, 1),    # tile 10: parts >= 86 are relu
}


def _emit_hidden_act(nc, ps, blk, a_new, tmp_pool, bmask_t, hbc_t):
    """Run a 4-m-tile block of WS-scaled pre-activations through the grouped
    activations into a_new, applying the raw hidden bias inside the ACT.

    ps:    AP (128, 4*CH) holding m-tiles blk*4..blk*4+3 side by side
    a_new: SBUF tile (128, KH, CH) fp8, m-tile m lives at [:, m, :]
    hbc_t: (128, KH) f32 per-partition raw biases, column m for m-tile m
    """
    for mloc in range(4):
        m = blk * 4 + mloc
        bias = hbc_t[:, m:m + 1]
        src = ps[:, mloc * CH:(mloc + 1) * CH]
        if m in _BOUNDARY:
            major, minor, mb = _BOUNDARY[m]
            nc.scalar.activation(a_new[:, m:m + 1, :], src, major,
                                 bias=bias, scale=IWS)
            t = tmp_pool.tile([128, CH], FP8, tag="btmp", bufs=2, name="btmp")
            nc.scalar.activation(t[:], src, minor, bias=bias, scale=IWS)
            nc.vector.copy_predicated(
                a_new[:, m:m + 1, :],
                bmask_t[:, mb * CH:(mb + 1) * CH], t[:])
        else:
            nc.scalar.activation(a_new[:, m:m + 1, :], src, _TILE_FUNC[m],
                                 bias=bias, scale=IWS)


def _build_nc():
    nc = bacc.Bacc("TRN2", target_bir_lowering=False, debug=False,
                   num_devices=N_CORES, dynamic_dma_scratch_size=2048)

    # all operands host-packed so each loads as one large contiguous DMA;
    # k-tile k of a weight lives at [:, k, :] so DoubleRow k-pairs are
    # adjacent in the middle dim
    xT = nc.dram_tensor("xT", [128, KI, BL], FP8, kind="ExternalInput").ap()
    wih = nc.dram_tensor("wih", [128, KI, NH], FP8, kind="ExternalInput").ap()
    whh = nc.dram_tensor("whh", [4 * 128, 4, NH], FP8,
                         kind="ExternalInput").ap()
    who = nc.dram_tensor("who", [128, KH, NO], FP8, kind="ExternalInput").ap()
    wio = nc.dram_tensor("wio", [128, KI, NO], FP8, kind="ExternalInput").ap()
    hbc = nc.dram_tensor("hbc", [128, KH], F32, kind="ExternalInput").ap()
    obc = nc.dram_tensor("obc", [128, KO], F32, kind="ExternalInput").ap()
    bmask = nc.dram_tensor("bmask", [128, 2 * CH], mybir.dt.uint8,
                           kind="ExternalInput").ap()
    outT = nc.dram_tensor("outT", [NO, BL], BF16, kind="ExternalOutput").ap()

    with tile.TileContext(nc) as tc:
        with tc.tile_pool(name="w", bufs=1) as wpool, \
             tc.tile_pool(name="act", bufs=1) as apool, \
             tc.tile_pool(name="ps", bufs=2, space="PSUM") as pspool, \
             tc.tile_pool(name="out", bufs=4) as opool:

            # ---- stage inputs across all three DMA trigger paths (SP +
            # ACT HWDGE queues, GpSimd software DGE), ordered first-needed-
            # first. The first x-proj PSUM group (m-tiles 0-3) only needs
            # wih cols 0:1024 of both k-pairs + the chunk-0 x columns, so
            # those small pieces lead the sync queue and the first matmul
            # can issue ~3us after queue start. ----
            wih_m = wpool.tile([128, KI, NH], FP8, tag="projA", name="wihm")
            x_m = wpool.tile([128, KI, BL], FP8, tag="x", name="xm")
            hbc_t = wpool.tile([128, KH], F32, tag="hbc")
            obc_t = wpool.tile([128, KO], F32, tag="obc")
            bmask_t = wpool.tile([128, 2 * CH], mybir.dt.uint8, tag="bmask")
            wio_m = wpool.tile([128, KI, NO], FP8, tag="wio", name="wiom")
            whh_m = [wpool.tile([128, 4, NH], FP8, tag=f"whhJ{J}",
                                name=f"whhJ{J}") for J in range(4)]
            nc.sync.dma_start(wih_m[:, 0:2, :], wih[:, 0:2, :])
            nc.sync.dma_start(wih_m[:, 2:4, :], wih[:, 2:4, :])
            nc.sync.dma_start(whh_m[0][:], whh[0:128])
            nc.sync.dma_start(whh_m[2][:], whh[2 * 128:3 * 128])
            nc.scalar.dma_start(x_m[:, 0:2, :], xT[:, 0:2, :])
            nc.scalar.dma_start(hbc_t[:], hbc[:])
            nc.scalar.dma_start(x_m[:, 2:4, :], xT[:, 2:4, :])
            nc.scalar.dma_start(bmask_t[:], bmask[:])
            nc.scalar.dma_start(whh_m[1][:], whh[128:2 * 128])
            nc.scalar.dma_start(obc_t[:], obc[:])
            nc.gpsimd.dma_start(wio_m[:], wio[:])
            nc.gpsimd.dma_start(whh_m[3][:], whh[3 * 128:4 * 128])

            def whh_pair(kp):      # lhsT k-pair AP for hh k-tiles 2kp,2kp+1
                J, j = divmod(2 * kp, 4)
                return whh_m[J], j

            # ---- per-chunk x-projection P and first-step activations ----
            P = {}
            A = {}
            for c in range(NCH):
                P[c] = apool.tile([128, KH * CH], BF16, tag=f"P{c}",
                                  name=f"P{c}")
                a1 = apool.tile([128, KH, CH], FP8, tag="A", bufs=3,
                                name=f"A1c{c}")
                for blk in range(4):
                    ps = pspool.tile([128, 4 * CH], F32, tag="ps", name="psb")
                    for kp in range(KI // 2):
                        for mloc in range(4):
                            m = blk * 4 + mloc
                            nc.tensor.matmul(
                                ps[:, mloc * CH:(mloc + 1) * CH],
                                wih_m[:, 2 * kp:2 * kp + 2,
                                      m * 128:(m + 1) * 128],
                                x_m[:, 2 * kp:2 * kp + 2,
                                    c * CH:(c + 1) * CH],
                                start=(kp == 0), stop=(kp == KI // 2 - 1),
                                perf_mode=DR)
                    # P holds the raw WS-scaled x-projection (bias is applied
                    # inside the ACTs); a single copy frees the PSUM slot
                    nc.vector.tensor_copy(
                        P[c][:, blk * 4 * CH:(blk + 1) * 4 * CH], ps[:])
                    _emit_hidden_act(nc, P[c][:, blk * 4 * CH:(blk + 1) * 4 * CH],
                                     blk, a1, opool, bmask_t, hbc_t)
                A[c] = a1

            # ---- whh-independent output x-projection (fills the window
            # while the whh load is still in flight) ----
            outx = {}
            for c in range(NCH):
                outx[c] = apool.tile([128, KO * CH], BF16, tag=f"outx{c}",
                                     name=f"outx{c}")
                ps = pspool.tile([128, 4 * CH], F32, tag="ps", name="psb")
                for kp in range(KI // 2):
                    for mo in range(KO):
                        nc.tensor.matmul(
                            ps[:, mo * CH:(mo + 1) * CH],
                            wio_m[:, 2 * kp:2 * kp + 2,
                                  mo * 128:(mo + 1) * 128],
                            x_m[:, 2 * kp:2 * kp + 2, c * CH:(c + 1) * CH],
                            start=(kp == 0), stop=(kp == KI // 2 - 1),
                            perf_mode=DR)
                nc.vector.tensor_copy(outx[c][:], ps[:])

            # ---- recurrent steps 2..4 ----
            def hh_step(c, s):
                a_new = apool.tile([128, KH, CH], FP8, tag="A", bufs=3,
                                   name=f"A{s + 2}c{c}")
                for blk in range(4):
                    ps = pspool.tile([128, 4 * CH], F32, tag="ps", name="psb")
                    for kp in range(KH // 2):
                        wt, j = whh_pair(kp)
                        for mloc in range(4):
                            m = blk * 4 + mloc
                            nc.tensor.matmul(
                                ps[:, mloc * CH:(mloc + 1) * CH],
                                wt[:, j:j + 2, m * 128:(m + 1) * 128],
                                A[c][:, 2 * kp:2 * kp + 2, :],
                                start=(kp == 0), stop=(kp == KH // 2 - 1),
                                perf_mode=DR)
                    # pre = psum + P into an SBUF temp: a single PSUM read
                    # frees the bank; ACT then runs off SBUF
                    tmp = opool.tile([128, 4 * CH], F32, tag="pre", bufs=2,
                                     name="pre")
                    nc.vector.tensor_add(
                        tmp[:], ps[:], P[c][:, blk * 4 * CH:(blk + 1) * 4 * CH])
                    _emit_hidden_act(nc, tmp, blk, a_new, opool, bmask_t,
                                     hbc_t)
                A[c] = a_new

            for s in range(N_STEPS - 2):
                for c in range(NCH):
                    hh_step(c, s)
            hh_step(0, N_STEPS - 2)  # chunk 1's final step emitted after who

            # ---- output layer (who reuses the wih slot); chunk 0's
            # output overlaps chunk 1's final hh step ----
            who_m = wpool.tile([128, KH, NO], FP8, tag="projA", name="whom")
            nc.sync.dma_start(who_m[:], who[:])

            def out_chunk(c):
                for mo in range(KO):
                    pso = pspool.tile([128, CH], F32, tag="ps", name="pso")
                    oap = pso[:]
                    for kp in range(KH // 2):
                        nc.tensor.matmul(
                            oap,
                            who_m[:, 2 * kp:2 * kp + 2,
                                  mo * 128:(mo + 1) * 128],
                            A[c][:, 2 * kp:2 * kp + 2, :],
                            start=(kp == 0), stop=(kp == KH // 2 - 1),
                            perf_mode=DR)
                    to = opool.tile([128, CH], F32, tag="preo", bufs=2,
                                    name="preo")
                    nc.vector.tensor_add(
                        to[:], oap, outx[c][:, mo * CH:(mo + 1) * CH])
                    o = opool.tile([128, CH], BF16, tag="o", bufs=2, name="o")
                    nc.scalar.activation(o[:], to[:], AF.Sigmoid,
                                         bias=obc_t[:, mo:mo + 1], scale=IWS)
                    nc.sync.dma_start(
                        outT[mo * 128:(mo + 1) * 128, c * CH:(c + 1) * CH],
                        o[:])

            hh_step(1, N_STEPS - 2)
            out_chunk(0)
            out_chunk(1)

    nc.compile()
    return nc


_NC_CACHE = None


def _get_nc():
    global _NC_CACHE
    if _NC_CACHE is None:
        _NC_CACHE = _build_nc()
    return _NC_CACHE


def _make_bmask():
    m = np.zeros((128, 2 * CH), np.uint8)
    m[:_B1 - (_B1 // 128) * 128, 0:CH] = 1          # tile 5: parts < 43 tanh
    m[_B2 - (_B2 // 128) * 128:, CH:2 * CH] = 1     # tile 10: parts >= 86 relu
    return m


def _q8(a):
    return np.clip(a, -240.0, 240.0).astype(E4)


def _prep_in_maps(inputs):
    x = np.asarray(inputs["inputs"], np.float32)
    hr = np.asarray(inputs["hidden_responses"], np.float32)[PERM]
    hb = np.asarray(inputs["hidden_biases"], np.float32)[PERM]
    orr = np.asarray(inputs["output_responses"], np.float32)
    ob = np.asarray(inputs["output_biases"], np.float32)

    wih_s = WS * (hr[:, None] * np.asarray(inputs["input_to_hidden"], np.float32)[PERM]).T
    whh_s = WS * (hr[:, None] *
                  np.asarray(inputs["hidden_to_hidden"], np.float32)[PERM][:, PERM]).T
    who_s = WS * (orr[:, None] *
                  np.asarray(inputs["hidden_to_output"], np.float32)[:, PERM]).T
    wio_s = WS * (orr[:, None] * np.asarray(inputs["input_to_output"], np.float32)).T

    def pack(w, ktiles):     # (ktiles*128, C) -> (128, ktiles, C)
        c = w.shape[1]
        return np.ascontiguousarray(
            w.reshape(ktiles, 128, c).transpose(1, 0, 2))

    # who: [p, kk, c] = who_s[kk*128 + p, c]
    who_p = pack(who_s, KH)
    # whh: [J*128 + p, j, col] = whh_s[(4J+j)*128 + p, col]
    whh_p = np.ascontiguousarray(
        whh_s.reshape(4, 4, 128, NH).transpose(0, 2, 1, 3).reshape(
            4 * 128, 4, NH))

    shared = {
        "wih": _q8(pack(wih_s, KI)),
        "whh": _q8(whh_p),
        "who": _q8(who_p),
        "wio": _q8(pack(wio_s, KI)),
        "hbc": np.ascontiguousarray(hb.reshape(KH, 128).T),
        "obc": np.ascontiguousarray(ob.reshape(KO, 128).T),
        "bmask": _make_bmask(),
    }
    in_maps = []
    for c in range(N_CORES):
        m = dict(shared)
        m["xT"] = _q8(pack(np.ascontiguousarray(x[c * BL:(c + 1) * BL].T), KI))
        in_maps.append(m)
    return in_maps


def _run(inputs, trace=False, tmpdir=None):
    nc = _get_nc()
    in_maps = _prep_in_maps(inputs)
    res = run_bass_kernel_spmd(nc, in_maps, core_ids=list(range(N_CORES)),
                               trace=trace, tmpdir=tmpdir)
    out = np.empty((B, NO), np.float32)
    for c in range(N_CORES):
        out[c * BL:(c + 1) * BL] = res.results[c]["outT"].T.astype(np.float32)
    return out, res


def kernel(**inputs) -> np.ndarray:
    out, _ = _run(inputs, trace=False)
    return out


if __name__ == "__main__":
    rng = np.random.default_rng(0)
    ins = {
        "inputs": rng.standard_normal((B, NI), dtype=np.float32),
        "input_to_hidden": rng.standard_normal((NH, NI), dtype=np.float32) * 0.02,
        "hidden_to_hidden": rng.standard_normal((NH, NH), dtype=np.float32) * 0.02,
        "output_to_hidden": rng.standard_normal((NH, NO), dtype=np.float32) * 0.02,
        "input_to_output": rng.standard_normal((NO, NI), dtype=np.float32) * 0.02,
        "hidden_to_output": rng.standard_normal((NO, NH), dtype=np.float32) * 0.02,
        "output_to_output": rng.standard_normal((NO, NO), dtype=np.float32) * 0.02,
        "hidden_responses": rng.standard_normal(NH, dtype=np.float32) * 0.1 + 1.0,
        "hidden_biases": rng.standard_normal(NH, dtype=np.float32) * 0.1,
        "output_responses": rng.standard_normal(NO, dtype=np.float32) * 0.1 + 1.0,
        "output_biases": rng.standard_normal(NO, dtype=np.float32) * 0.1,
    }
    out = kernel(**ins)
    print("kernel output", out.shape, out.dtype, out[:2, :4])


# revision 21
# speedup vs baseline: 1.0075x; 1.0075x over previous
"""Trainium2 Bass kernel for a 4-step differentiable recurrent net forward pass.

Reference computation (B=8192, NI=512, NH=2048, NO=512, 4 steps):
    activs = 0; outputs = 0
    repeat 4x:  pre = hr * (x @ Wih.T + activs @ Whh.T + outputs @ Woh.T) + hb
                activs = per_neuron_act(pre)        # tanh/sigmoid/relu by i%3
    out = sigmoid(or * (x @ Wio.T + outputs @ Woo.T + activs @ Who.T) + ob)

`outputs` is never written inside the loop, so the Woh/Woo terms vanish and
the x-projection P = hr*(x@Wih.T)+hb is loop-invariant (computed once).

Strategy: data-parallel on batch across 8 cores (1024 rows each). On-core
everything is feature-major (features on SBUF partitions, batch on the free
axis), so each matmul is W_tile.T @ X^T with stationary weights. All matmuls
run in fp8 e4m3 with DoubleRow perf mode (two k-tiles per instruction, 2x
MAC throughput). Weights are scaled by S=256 host-side so their ~0.02-scale
values sit in e4m3's normal range; the 1/S is folded into the activation
instruction's input scale. Activations are quantized to e4m3 unscaled (they
are O(1)). PSUM accumulates in fp32 throughout, so only operand quantization
loses precision (~1.3e-2 rel err on the final sigmoid outputs).
Host-side prep: hidden neurons are permuted so the three activation groups
are contiguous, hr/or are folded into the weight matrices, weights are
packed so each loads as one large contiguous DMA, and hb/ob are applied as
per-partition bias APs.
"""

import os

import numpy as np
import ml_dtypes

import concourse.bass as bass
import concourse.tile as tile
from concourse import bacc, mybir
from concourse.bass_utils import run_bass_kernel_spmd

B, NI, NH, NO = 8192, 512, 2048, 512
N_STEPS = 4
N_CORES = 8
BL = B // N_CORES          # batch rows per core
CH = 512                   # batch chunk (one PSUM bank of fp32)
NCH = BL // CH             # 2 chunks per core
KI = NI // 128             # 4 k-tiles over inputs
KH = NH // 128             # 16 k/m-tiles over hidden
KO = NO // 128             # 4 m-tiles over outputs

FP8 = mybir.dt.float8e4
BF16 = mybir.dt.bfloat16
F32 = mybir.dt.float32
AF = mybir.ActivationFunctionType
DR = mybir.MatmulPerfMode.DoubleRow
E4 = ml_dtypes.float8_e4m3

WS = 256.0                 # weight scale into fp8 range
IWS = 1.0 / WS             # folded back out at activation time

# hidden neurons regrouped as [all tanh | all sigmoid | all relu]
_idx = np.arange(NH)
PERM = np.concatenate([_idx[_idx % 3 == 0], _idx[_idx % 3 == 1], _idx[_idx % 3 == 2]])
_B1 = int((_idx % 3 == 0).sum())           # 683
_B2 = _B1 + int((_idx % 3 == 1).sum())     # 1366

# per m-tile: the single activation function, or None for the two mixed tiles
_TILE_FUNC = []
for _m in range(KH):
    _lo, _hi = _m * 128, (_m + 1) * 128
    _fs = set()
    for _f, _a, _b in ((AF.Tanh, 0, _B1), (AF.Sigmoid, _B1, _B2), (AF.Relu, _B2, NH)):
        if max(_lo, _a) < min(_hi, _b):
            _fs.add(_f)
    _TILE_FUNC.append(_fs.pop() if len(_fs) == 1 else None)

# mixed tiles: (major_func applied everywhere, minor_func, mask column block)
# partition sub-ranges must be 32-aligned on TRN2, so the minority strip is
# fixed up with a full-tile ACT + copy_predicated against a {0,1} mask
_BOUNDARY = {
    _B1 // 128: (AF.Sigmoid, AF.Tanh, 0),    # tile 5: parts < 43 are tanh
    _B2 // 128: (AF.Sigmoid, AF.Relu, 1),    # tile 10: parts >= 86 are relu
}


def _emit_hidden_act(nc, ps, blk, a_new, tmp_pool, bmask_t, hbc_t):
    """Run a 4-m-tile block of WS-scaled pre-activations through the grouped
    activations into a_new, applying the raw hidden bias inside the ACT.

    ps:    AP (128, 4*CH) holding m-tiles blk*4..blk*4+3 side by side
    a_new: SBUF tile (128, KH, CH) fp8, m-tile m lives at [:, m, :]
    hbc_t: (128, KH) f32 per-partition raw biases, column m for m-tile m
    """
    for mloc in range(4):
        m = blk * 4 + mloc
        bias = hbc_t[:, m:m + 1]
        src = ps[:, mloc * CH:(mloc + 1) * CH]
        if m in _BOUNDARY:
            major, minor, mb = _BOUNDARY[m]
            nc.scalar.activation(a_new[:, m:m + 1, :], src, major,
                                 bias=bias, scale=IWS)
            t = tmp_pool.tile([128, CH], FP8, tag="btmp", bufs=2, name="btmp")
            nc.scalar.activation(t[:], src, minor, bias=bias, scale=IWS)
            nc.vector.copy_predicated(
                a_new[:, m:m + 1, :],
                bmask_t[:, mb * CH:(mb + 1) * CH], t[:])
        else:
            nc.scalar.activation(a_new[:, m:m + 1, :], src, _TILE_FUNC[m],
                                 bias=bias, scale=IWS)


def _build_nc():
    nc = bacc.Bacc("TRN2", target_bir_lowering=False, debug=False,
                   num_devices=N_CORES, dynamic_dma_scratch_size=2048)

    # all operands host-packed so each loads as one large contiguous DMA;
    # k-tile k of a weight lives at [:, k, :] so DoubleRow k-pairs are
    # adjacent in the middle dim
    xT = nc.dram_tensor("xT", [128, KI, BL], FP8, kind="ExternalInput").ap()
    wih = nc.dram_tensor("wih", [128, KI, NH], FP8, kind="ExternalInput").ap()
    whh = nc.dram_tensor("whh", [4 * 128, 4, NH], FP8,
                         kind="ExternalInput").ap()
    who = nc.dram_tensor("who", [128, KH, NO], FP8, kind="ExternalInput").ap()
    wio = nc.dram_tensor("wio", [128, KI, NO], FP8, kind="ExternalInput").ap()
    hbc = nc.dram_tensor("hbc", [128, KH], F32, kind="ExternalInput").ap()
    obc = nc.dram_tensor("obc", [128, KO], F32, kind="ExternalInput").ap()
    bmask = nc.dram_tensor("bmask", [128, 2 * CH], mybir.dt.uint8,
                           kind="ExternalInput").ap()
    outT = nc.dram_tensor("outT", [NO, BL], BF16, kind="ExternalOutput").ap()

    with tile.TileContext(nc) as tc:
        with tc.tile_pool(name="w", bufs=1) as wpool, \
             tc.tile_pool(name="act", bufs=1) as apool, \
             tc.tile_pool(name="ps", bufs=2, space="PSUM") as pspool, \
             tc.tile_pool(name="out", bufs=4) as opool:

            # ---- stage inputs across all three DMA trigger paths (SP +
            # ACT HWDGE queues, GpSimd software DGE), ordered first-needed-
            # first. The first x-proj PSUM group (m-tiles 0-3) only needs
            # wih cols 0:1024 of both k-pairs + the chunk-0 x columns, so
            # those small pieces lead the sync queue and the first matmul
            # can issue ~3us after queue start. ----
            wih_m = wpool.tile([128, KI, NH], FP8, tag="projA", name="wihm")
            x_m = wpool.tile([128, KI, BL], FP8, tag="x", name="xm")
            hbc_t = wpool.tile([128, KH], F32, tag="hbc")
            obc_t = wpool.tile([128, KO], F32, tag="obc")
            bmask_t = wpool.tile([128, 2 * CH], mybir.dt.uint8, tag="bmask")
            wio_m = wpool.tile([128, KI, NO], FP8, tag="wio", name="wiom")
            whh_m = [wpool.tile([128, 4, NH], FP8, tag=f"whhJ{J}",
                                name=f"whhJ{J}") for J in range(4)]
            nc.sync.dma_start(wih_m[:, 0:2, :], wih[:, 0:2, :])
            nc.sync.dma_start(wih_m[:, 2:4, :], wih[:, 2:4, :])
            nc.scalar.dma_start(x_m[:, 0:2, :], xT[:, 0:2, :])
            nc.scalar.dma_start(hbc_t[:], hbc[:])
            nc.scalar.dma_start(x_m[:, 2:4, :], xT[:, 2:4, :])
            nc.scalar.dma_start(bmask_t[:], bmask[:])
            nc.gpsimd.dma_start(wio_m[:], wio[:])
            # whh J0-J2 land in m-column pieces, m-major, so the first hh
            # step's m-block b only waits on pieces 0..b and DMA pipelines
            # against the m-block progression; J3 rides the gpsimd software
            # DGE as one contiguous transfer (strided gpsimd DMAs complete
            # their semaphore unreliably)
            nc.gpsimd.dma_start(whh_m[3][:], whh[3 * 128:4 * 128])
            for mb in range(4):
                lo, hi = mb * 512, (mb + 1) * 512
                for J, eng in ((0, nc.sync), (1, nc.scalar), (2, nc.sync)):
                    eng.dma_start(whh_m[J][:, :, lo:hi],
                                  whh[J * 128:(J + 1) * 128, :, lo:hi])
            nc.scalar.dma_start(obc_t[:], obc[:])

            def whh_pair(kp):      # lhsT k-pair AP for hh k-tiles 2kp,2kp+1
                J, j = divmod(2 * kp, 4)
                return whh_m[J], j

            # ---- per-chunk x-projection P and first-step activations ----
            P = {}
            A = {}
            for c in range(NCH):
                P[c] = apool.tile([128, KH * CH], BF16, tag=f"P{c}",
                                  name=f"P{c}")
                a1 = apool.tile([128, KH, CH], FP8, tag="A", bufs=3,
                                name=f"A1c{c}")
                for blk in range(4):
                    ps = pspool.tile([128, 4 * CH], F32, tag="ps", name="psb")
                    for kp in range(KI // 2):
                        for mloc in range(4):
                            m = blk * 4 + mloc
                            nc.tensor.matmul(
                                ps[:, mloc * CH:(mloc + 1) * CH],
                                wih_m[:, 2 * kp:2 * kp + 2,
                                      m * 128:(m + 1) * 128],
                                x_m[:, 2 * kp:2 * kp + 2,
                                    c * CH:(c + 1) * CH],
                                start=(kp == 0), stop=(kp == KI // 2 - 1),
                                perf_mode=DR)
                    # P holds the raw WS-scaled x-projection (bias is applied
                    # inside the ACTs); a single copy frees the PSUM slot
                    nc.vector.tensor_copy(
                        P[c][:, blk * 4 * CH:(blk + 1) * 4 * CH], ps[:])
                    _emit_hidden_act(nc, P[c][:, blk * 4 * CH:(blk + 1) * 4 * CH],
                                     blk, a1, opool, bmask_t, hbc_t)
                A[c] = a1

            # ---- whh-independent output x-projection (fills the window
            # while the whh load is still in flight) ----
            outx = {}
            for c in range(NCH):
                outx[c] = apool.tile([128, KO * CH], BF16, tag=f"outx{c}",
                                     name=f"outx{c}")
                ps = pspool.tile([128, 4 * CH], F32, tag="ps", name="psb")
                for kp in range(KI // 2):
                    for mo in range(KO):
                        nc.tensor.matmul(
                            ps[:, mo * CH:(mo + 1) * CH],
                            wio_m[:, 2 * kp:2 * kp + 2,
                                  mo * 128:(mo + 1) * 128],
                            x_m[:, 2 * kp:2 * kp + 2, c * CH:(c + 1) * CH],
                            start=(kp == 0), stop=(kp == KI // 2 - 1),
                            perf_mode=DR)
                nc.vector.tensor_copy(outx[c][:], ps[:])

            # ---- recurrent steps 2..4 ----
            def hh_step(c, s):
                a_new = apool.tile([128, KH, CH], FP8, tag="A", bufs=3,
                                   name=f"A{s + 2}c{c}")
                for blk in range(4):
                    ps = pspool.tile([128, 4 * CH], F32, tag="ps", name="psb")
                    for kp in range(KH // 2):
                        wt, j = whh_pair(kp)
                        for mloc in range(4):
                            m = blk * 4 + mloc
                            nc.tensor.matmul(
                                ps[:, mloc * CH:(mloc + 1) * CH],
                                wt[:, j:j + 2, m * 128:(m + 1) * 128],
                                A[c][:, 2 * kp:2 * kp + 2, :],
                                start=(kp == 0), stop=(kp == KH // 2 - 1),
                                perf_mode=DR)
                    # pre = psum + P into an SBUF temp: a single PSUM read
                    # frees the bank; ACT then runs off SBUF
                    tmp = opool.tile([128, 4 * CH], F32, tag="pre", bufs=2,
                                     name="pre")
                    nc.vector.tensor_add(
                        tmp[:], ps[:], P[c][:, blk * 4 * CH:(blk + 1) * 4 * CH])
                    _emit_hidden_act(nc, tmp, blk, a_new, opool, bmask_t,
                                     hbc_t)
                A[c] = a_new

            for s in range(N_STEPS - 2):
                for c in range(NCH):
                    hh_step(c, s)
            hh_step(0, N_STEPS - 2)  # chunk 1's final step emitted after who

            # ---- output layer (who reuses the wih slot); chunk 0's
            # output overlaps chunk 1's final hh step ----
            who_m = wpool.tile([128, KH, NO], FP8, tag="projA", name="whom")
            nc.sync.dma_start(who_m[:], who[:])

            def out_chunk(c):
                for mo in range(KO):
                    pso = pspool.tile([128, CH], F32, tag="ps", name="pso")
                    oap = pso[:]
                    for kp in range(KH // 2):
                        nc.tensor.matmul(
                            oap,
                            who_m[:, 2 * kp:2 * kp + 2,
                                  mo * 128:(mo + 1) * 128],
                            A[c][:, 2 * kp:2 * kp + 2, :],
                            start=(kp == 0), stop=(kp == KH // 2 - 1),
                            perf_mode=DR)
                    to = opool.tile([128, CH], F32, tag="preo", bufs=2,
                                    name="preo")
                    nc.vector.tensor_add(
                        to[:], oap, outx[c][:, mo * CH:(mo + 1) * CH])
                    o = opool.tile([128, CH], BF16, tag="o", bufs=2, name="o")
                    nc.scalar.activation(o[:], to[:], AF.Sigmoid,
                                         bias=obc_t[:, mo:mo + 1], scale=IWS)
                    nc.sync.dma_start(
                        outT[mo * 128:(mo + 1) * 128, c * CH:(c + 1) * CH],
                        o[:])

            hh_step(1, N_STEPS - 2)
            out_chunk(0)
            out_chunk(1)

    nc.compile()
    return nc


_NC_CACHE = None


def _get_nc():
    global _NC_CACHE
    if _NC_CACHE is None:
        _NC_CACHE = _build_nc()
    return _NC_CACHE


def _make_bmask():
    m = np.zeros((128, 2 * CH), np.uint8)
    m[:_B1 - (_B1 // 128) * 128, 0:CH] = 1          # tile 5: parts < 43 tanh
    m[_B2 - (_B2 // 128) * 128:, CH:2 * CH] = 1     # tile 10: parts >= 86 relu
    return m


def _q8(a):
    return np.clip(a, -240.0, 240.0).astype(E4)


def _prep_in_maps(inputs):
    x = np.asarray(inputs["inputs"], np.float32)
    hr = np.asarray(inputs["hidden_responses"], np.float32)[PERM]
    hb = np.asarray(inputs["hidden_biases"], np.float32)[PERM]
    orr = np.asarray(inputs["output_responses"], np.float32)
    ob = np.asarray(inputs["output_biases"], np.float32)

    wih_s = WS * (hr[:, None] * np.asarray(inputs["input_to_hidden"], np.float32)[PERM]).T
    whh_s = WS * (hr[:, None] *
                  np.asarray(inputs["hidden_to_hidden"], np.float32)[PERM][:, PERM]).T
    who_s = WS * (orr[:, None] *
                  np.asarray(inputs["hidden_to_output"], np.float32)[:, PERM]).T
    wio_s = WS * (orr[:, None] * np.asarray(inputs["input_to_output"], np.float32)).T

    def pack(w, ktiles):     # (ktiles*128, C) -> (128, ktiles, C)
        c = w.shape[1]
        return np.ascontiguousarray(
            w.reshape(ktiles, 128, c).transpose(1, 0, 2))

    # who: [p, kk, c] = who_s[kk*128 + p, c]
    who_p = pack(who_s, KH)
    # whh: [J*128 + p, j, col] = whh_s[(4J+j)*128 + p, col]
    whh_p = np.ascontiguousarray(
        whh_s.reshape(4, 4, 128, NH).transpose(0, 2, 1, 3).reshape(
            4 * 128, 4, NH))

    shared = {
        "wih": _q8(pack(wih_s, KI)),
        "whh": _q8(whh_p),
        "who": _q8(who_p),
        "wio": _q8(pack(wio_s, KI)),
        "hbc": np.ascontiguousarray(hb.reshape(KH, 128).T),
        "obc": np.ascontiguousarray(ob.reshape(KO, 128).T),
        "bmask": _make_bmask(),
    }
    in_maps = []
    for c in range(N_CORES):
        m = dict(shared)
        m["xT"] = _q8(pack(np.ascontiguousarray(x[c * BL:(c + 1) * BL].T), KI))
        in_maps.append(m)
    return in_maps


def _run(inputs, trace=False, tmpdir=None):
    nc = _get_nc()
    in_maps = _prep_in_maps(inputs)
    res = run_bass_kernel_spmd(nc, in_maps, core_ids=list(range(N_CORES)),
                               trace=trace, tmpdir=tmpdir)
    out = np.empty((B, NO), np.float32)
    for c in range(N_CORES):
        out[c * BL:(c + 1) * BL] = res.results[c]["outT"].T.astype(np.float32)
    return out, res


def kernel(**inputs) -> np.ndarray:
    out, _ = _run(inputs, trace=False)
    return out


if __name__ == "__main__":
    rng = np.random.default_rng(0)
    ins = {
        "inputs": rng.standard_normal((B, NI), dtype=np.float32),
        "input_to_hidden": rng.standard_normal((NH, NI), dtype=np.float32) * 0.02,
        "hidden_to_hidden": rng.standard_normal((NH, NH), dtype=np.float32) * 0.02,
        "output_to_hidden": rng.standard_normal((NH, NO), dtype=np.float32) * 0.02,
        "input_to_output": rng.standard_normal((NO, NI), dtype=np.float32) * 0.02,
        "hidden_to_output": rng.standard_normal((NO, NH), dtype=np.float32) * 0.02,
        "output_to_output": rng.standard_normal((NO, NO), dtype=np.float32) * 0.02,
        "hidden_responses": rng.standard_normal(NH, dtype=np.float32) * 0.1 + 1.0,
        "hidden_biases": rng.standard_normal(NH, dtype=np.float32) * 0.1,
        "output_responses": rng.standard_normal(NO, dtype=np.float32) * 0.1 + 1.0,
        "output_biases": rng.standard_normal(NO, dtype=np.float32) * 0.1,
    }
    out = kernel(**ins)
    print("kernel output", out.shape, out.dtype, out[:2, :4])


# revision 23
# speedup vs baseline: 1.0151x; 1.0076x over previous
"""Trainium2 Bass kernel for a 4-step differentiable recurrent net forward pass.

Reference computation (B=8192, NI=512, NH=2048, NO=512, 4 steps):
    activs = 0; outputs = 0
    repeat 4x:  pre = hr * (x @ Wih.T + activs @ Whh.T + outputs @ Woh.T) + hb
                activs = per_neuron_act(pre)        # tanh/sigmoid/relu by i%3
    out = sigmoid(or * (x @ Wio.T + outputs @ Woo.T + activs @ Who.T) + ob)

`outputs` is never written inside the loop, so the Woh/Woo terms vanish and
the x-projection P = hr*(x@Wih.T)+hb is loop-invariant (computed once).

Strategy: data-parallel on batch across 8 cores (1024 rows each). On-core
everything is feature-major (features on SBUF partitions, batch on the free
axis), so each matmul is W_tile.T @ X^T with stationary weights. All matmuls
run in fp8 e4m3 with DoubleRow perf mode (two k-tiles per instruction, 2x
MAC throughput). Weights are scaled by S=256 host-side so their ~0.02-scale
values sit in e4m3's normal range; the 1/S is folded into the activation
instruction's input scale. Activations are quantized to e4m3 unscaled (they
are O(1)). PSUM accumulates in fp32 throughout, so only operand quantization
loses precision (~1.3e-2 rel err on the final sigmoid outputs).
Host-side prep: hidden neurons are permuted so the three activation groups
are contiguous, hr/or are folded into the weight matrices, weights are
packed so each loads as one large contiguous DMA, and hb/ob are applied as
per-partition bias APs.
"""

import os

import numpy as np
import ml_dtypes

import concourse.bass as bass
import concourse.tile as tile
from concourse import bacc, mybir
from concourse.bass_utils import run_bass_kernel_spmd

B, NI, NH, NO = 8192, 512, 2048, 512
N_STEPS = 4
N_CORES = 8
BL = B // N_CORES          # batch rows per core
CH = 512                   # batch chunk (one PSUM bank of fp32)
NCH = BL // CH             # 2 chunks per core
KI = NI // 128             # 4 k-tiles over inputs
KH = NH // 128             # 16 k/m-tiles over hidden
KO = NO // 128             # 4 m-tiles over outputs

FP8 = mybir.dt.float8e4
BF16 = mybir.dt.bfloat16
F32 = mybir.dt.float32
AF = mybir.ActivationFunctionType
DR = mybir.MatmulPerfMode.DoubleRow
E4 = ml_dtypes.float8_e4m3

WS = 256.0                 # weight scale into fp8 range
IWS = 1.0 / WS             # folded back out at activation time

# hidden neurons regrouped as [all tanh | all sigmoid | all relu]
_idx = np.arange(NH)
PERM = np.concatenate([_idx[_idx % 3 == 0], _idx[_idx % 3 == 1], _idx[_idx % 3 == 2]])
_B1 = int((_idx % 3 == 0).sum())           # 683
_B2 = _B1 + int((_idx % 3 == 1).sum())     # 1366

# per m-tile: the single activation function, or None for the two mixed tiles
_TILE_FUNC = []
for _m in range(KH):
    _lo, _hi = _m * 128, (_m + 1) * 128
    _fs = set()
    for _f, _a, _b in ((AF.Tanh, 0, _B1), (AF.Sigmoid, _B1, _B2), (AF.Relu, _B2, NH)):
        if max(_lo, _a) < min(_hi, _b):
            _fs.add(_f)
    _TILE_FUNC.append(_fs.pop() if len(_fs) == 1 else None)

# mixed tiles: (major_func applied everywhere, minor_func, mask column block)
# partition sub-ranges must be 32-aligned on TRN2, so the minority strip is
# fixed up with a full-tile ACT + copy_predicated against a {0,1} mask
_BOUNDARY = {
    _B1 // 128: (AF.Sigmoid, AF.Tanh, 0),    # tile 5: parts < 43 are tanh
    _B2 // 128: (AF.Sigmoid, AF.Relu, 1),    # tile 10: parts >= 86 are relu
}


def _emit_hidden_act(nc, ps, blk, a_new, tmp_pool, bmask_t, hbc_t):
    """Run a 4-m-tile block of WS-scaled pre-activations through the grouped
    activations into a_new, applying the raw hidden bias inside the ACT.

    ps:    AP (128, 4*CH) holding m-tiles blk*4..blk*4+3 side by side
    a_new: SBUF tile (128, KH, CH) fp8, m-tile m lives at [:, m, :]
    hbc_t: (128, KH) f32 per-partition raw biases, column m for m-tile m
    """
    for mloc in range(4):
        m = blk * 4 + mloc
        bias = hbc_t[:, m:m + 1]
        src = ps[:, mloc * CH:(mloc + 1) * CH]
        if m in _BOUNDARY:
            major, minor, mb = _BOUNDARY[m]
            nc.scalar.activation(a_new[:, m:m + 1, :], src, major,
                                 bias=bias, scale=IWS)
            t = tmp_pool.tile([128, CH], FP8, tag="btmp", bufs=2, name="btmp")
            nc.scalar.activation(t[:], src, minor, bias=bias, scale=IWS)
            nc.vector.copy_predicated(
                a_new[:, m:m + 1, :],
                bmask_t[:, mb * CH:(mb + 1) * CH], t[:])
        else:
            nc.scalar.activation(a_new[:, m:m + 1, :], src, _TILE_FUNC[m],
                                 bias=bias, scale=IWS)


def _build_nc():
    nc = bacc.Bacc("TRN2", target_bir_lowering=False, debug=False,
                   num_devices=N_CORES, dynamic_dma_scratch_size=2048)

    # all operands host-packed so each loads as one large contiguous DMA;
    # k-tile k of a weight lives at [:, k, :] so DoubleRow k-pairs are
    # adjacent in the middle dim
    xT = nc.dram_tensor("xT", [128, KI, BL], FP8, kind="ExternalInput").ap()
    wih = nc.dram_tensor("wih", [128, KI, NH], FP8, kind="ExternalInput").ap()
    whh = nc.dram_tensor("whh", [4 * 128, 4, NH], FP8,
                         kind="ExternalInput").ap()
    who = nc.dram_tensor("who", [128, KH, NO], FP8, kind="ExternalInput").ap()
    wio = nc.dram_tensor("wio", [128, KI, NO], FP8, kind="ExternalInput").ap()
    hbc = nc.dram_tensor("hbc", [128, KH], F32, kind="ExternalInput").ap()
    obc = nc.dram_tensor("obc", [128, KO], F32, kind="ExternalInput").ap()
    bmask = nc.dram_tensor("bmask", [128, 2 * CH], mybir.dt.uint8,
                           kind="ExternalInput").ap()
    outT = nc.dram_tensor("outT", [NO, BL], BF16, kind="ExternalOutput").ap()

    with tile.TileContext(nc) as tc:
        with tc.tile_pool(name="w", bufs=1) as wpool, \
             tc.tile_pool(name="act", bufs=1) as apool, \
             tc.tile_pool(name="ps", bufs=2, space="PSUM") as pspool, \
             tc.tile_pool(name="out", bufs=4) as opool:

            # ---- stage inputs across all three DMA trigger paths (SP +
            # ACT HWDGE queues, GpSimd software DGE), ordered first-needed-
            # first. The first x-proj PSUM group (m-tiles 0-3) only needs
            # wih cols 0:1024 of both k-pairs + the chunk-0 x columns, so
            # those small pieces lead the sync queue and the first matmul
            # can issue ~3us after queue start. ----
            wih_m = wpool.tile([128, KI, NH], FP8, tag="projA", name="wihm")
            x_m = wpool.tile([128, KI, BL], FP8, tag="x", name="xm")
            hbc_t = wpool.tile([128, KH], F32, tag="hbc")
            obc_t = wpool.tile([128, KO], F32, tag="obc")
            bmask_t = wpool.tile([128, 2 * CH], mybir.dt.uint8, tag="bmask")
            wio_m = wpool.tile([128, KI, NO], FP8, tag="wio", name="wiom")
            whh_m = [wpool.tile([128, 4, NH], FP8, tag=f"whhJ{J}",
                                name=f"whhJ{J}") for J in range(4)]
            # ---- PE warmup: ~3.5us of dummy matmuls on an uninitialized
            # tile (no DMA dependency) so the HAM clock-gate is at 8/8
            # before the first real matmul issues (~12us in, behind DMA) ----
            warm_t = wpool.tile([128, 2, CH], FP8, tag="warm", name="warm")
            nc.gpsimd.memset(warm_t[:], 0.0)
            ps_w = pspool.tile([128, 4 * CH], F32, tag="ps", name="psw")
            for _w in range(16):
                nc.tensor.matmul(
                    ps_w[:, (_w % 4) * CH:(_w % 4 + 1) * CH],
                    warm_t[:, :, 0:128], warm_t[:],
                    start=True, stop=True, perf_mode=DR,
                    skip_group_check=True)

            nc.sync.dma_start(wih_m[:, 0:2, :], wih[:, 0:2, :])
            nc.sync.dma_start(wih_m[:, 2:4, :], wih[:, 2:4, :])
            nc.scalar.dma_start(x_m[:, 0:2, :], xT[:, 0:2, :])
            nc.scalar.dma_start(hbc_t[:], hbc[:])
            nc.scalar.dma_start(x_m[:, 2:4, :], xT[:, 2:4, :])
            nc.scalar.dma_start(bmask_t[:], bmask[:])
            nc.gpsimd.dma_start(wio_m[:], wio[:])
            # whh J0-J2 land in m-column pieces, m-major, so the first hh
            # step's m-block b only waits on pieces 0..b and DMA pipelines
            # against the m-block progression; J3 rides the gpsimd software
            # DGE as one contiguous transfer (strided gpsimd DMAs complete
            # their semaphore unreliably)
            nc.gpsimd.dma_start(whh_m[3][:], whh[3 * 128:4 * 128])
            for mb in range(4):
                lo, hi = mb * 512, (mb + 1) * 512
                for J, eng in ((0, nc.sync), (1, nc.scalar), (2, nc.sync)):
                    eng.dma_start(whh_m[J][:, :, lo:hi],
                                  whh[J * 128:(J + 1) * 128, :, lo:hi])
            nc.scalar.dma_start(obc_t[:], obc[:])

            def whh_pair(kp):      # lhsT k-pair AP for hh k-tiles 2kp,2kp+1
                J, j = divmod(2 * kp, 4)
                return whh_m[J], j

            # ---- per-chunk x-projection P and first-step activations ----
            P = {}
            A = {}
            for c in range(NCH):
                P[c] = apool.tile([128, KH * CH], BF16, tag=f"P{c}",
                                  name=f"P{c}")
                a1 = apool.tile([128, KH, CH], FP8, tag="A", bufs=3,
                                name=f"A1c{c}")
                for blk in range(4):
                    ps = pspool.tile([128, 4 * CH], F32, tag="ps", name="psb")
                    for kp in range(KI // 2):
                        for mloc in range(4):
                            m = blk * 4 + mloc
                            nc.tensor.matmul(
                                ps[:, mloc * CH:(mloc + 1) * CH],
                                wih_m[:, 2 * kp:2 * kp + 2,
                                      m * 128:(m + 1) * 128],
                                x_m[:, 2 * kp:2 * kp + 2,
                                    c * CH:(c + 1) * CH],
                                start=(kp == 0), stop=(kp == KI // 2 - 1),
                                perf_mode=DR)
                    # P holds the raw WS-scaled x-projection (bias is applied
                    # inside the ACTs); a single copy frees the PSUM slot
                    nc.vector.tensor_copy(
                        P[c][:, blk * 4 * CH:(blk + 1) * 4 * CH], ps[:])
                    _emit_hidden_act(nc, P[c][:, blk * 4 * CH:(blk + 1) * 4 * CH],
                                     blk, a1, opool, bmask_t, hbc_t)
                A[c] = a1

            # ---- whh-independent output x-projection (fills the window
            # while the whh load is still in flight) ----
            outx = {}
            for c in range(NCH):
                outx[c] = apool.tile([128, KO * CH], BF16, tag=f"outx{c}",
                                     name=f"outx{c}")
                ps = pspool.tile([128, 4 * CH], F32, tag="ps", name="psb")
                for kp in range(KI // 2):
                    for mo in range(KO):
                        nc.tensor.matmul(
                            ps[:, mo * CH:(mo + 1) * CH],
                            wio_m[:, 2 * kp:2 * kp + 2,
                                  mo * 128:(mo + 1) * 128],
                            x_m[:, 2 * kp:2 * kp + 2, c * CH:(c + 1) * CH],
                            start=(kp == 0), stop=(kp == KI // 2 - 1),
                            perf_mode=DR)
                nc.vector.tensor_copy(outx[c][:], ps[:])

            # ---- recurrent steps 2..4 ----
            def hh_step(c, s):
                a_new = apool.tile([128, KH, CH], FP8, tag="A", bufs=3,
                                   name=f"A{s + 2}c{c}")
                for blk in range(4):
                    ps = pspool.tile([128, 4 * CH], F32, tag="ps", name="psb")
                    for kp in range(KH // 2):
                        wt, j = whh_pair(kp)
                        for mloc in range(4):
                            m = blk * 4 + mloc
                            nc.tensor.matmul(
                                ps[:, mloc * CH:(mloc + 1) * CH],
                                wt[:, j:j + 2, m * 128:(m + 1) * 128],
                                A[c][:, 2 * kp:2 * kp + 2, :],
                                start=(kp == 0), stop=(kp == KH // 2 - 1),
                                perf_mode=DR)
                    # pre = psum + P into an SBUF temp: a single PSUM read
                    # frees the bank; ACT then runs off SBUF
                    tmp = opool.tile([128, 4 * CH], F32, tag="pre", bufs=2,
                                     name="pre")
                    nc.vector.tensor_add(
                        tmp[:], ps[:], P[c][:, blk * 4 * CH:(blk + 1) * 4 * CH])
                    _emit_hidden_act(nc, tmp, blk, a_new, opool, bmask_t,
                                     hbc_t)
                A[c] = a_new

            for s in range(N_STEPS - 2):
                for c in range(NCH):
                    hh_step(c, s)
            hh_step(0, N_STEPS - 2)  # chunk 1's final step emitted after who

            # ---- output layer (who reuses the wih slot); chunk 0's
            # output overlaps chunk 1's final hh step ----
            who_m = wpool.tile([128, KH, NO], FP8, tag="projA", name="whom")
            nc.sync.dma_start(who_m[:], who[:])

            def out_chunk(c):
                for mo in range(KO):
                    pso = pspool.tile([128, CH], F32, tag="ps", name="pso")
                    oap = pso[:]
                    for kp in range(KH // 2):
                        nc.tensor.matmul(
                            oap,
                            who_m[:, 2 * kp:2 * kp + 2,
                                  mo * 128:(mo + 1) * 128],
                            A[c][:, 2 * kp:2 * kp + 2, :],
                            start=(kp == 0), stop=(kp == KH // 2 - 1),
                            perf_mode=DR)
                    to = opool.tile([128, CH], F32, tag="preo", bufs=2,
                                    name="preo")
                    nc.vector.tensor_add(
                        to[:], oap, outx[c][:, mo * CH:(mo + 1) * CH])
                    o = opool.tile([128, CH], BF16, tag="o", bufs=2, name="o")
                    nc.scalar.activation(o[:], to[:], AF.Sigmoid,
                                         bias=obc_t[:, mo:mo + 1], scale=IWS)
                    nc.sync.dma_start(
                        outT[mo * 128:(mo + 1) * 128, c * CH:(c + 1) * CH],
                        o[:])

            hh_step(1, N_STEPS - 2)
            out_chunk(0)
            out_chunk(1)

    nc.compile()
    return nc


_NC_CACHE = None


def _get_nc():
    global _NC_CACHE
    if _NC_CACHE is None:
        _NC_CACHE = _build_nc()
    return _NC_CACHE


def _make_bmask():
    m = np.zeros((128, 2 * CH), np.uint8)
    m[:_B1 - (_B1 // 128) * 128, 0:CH] = 1          # tile 5: parts < 43 tanh
    m[_B2 - (_B2 // 128) * 128:, CH:2 * CH] = 1     # tile 10: parts >= 86 relu
    return m


def _q8(a):
    return np.clip(a, -240.0, 240.0).astype(E4)


def _prep_in_maps(inputs):
    x = np.asarray(inputs["inputs"], np.float32)
    hr = np.asarray(inputs["hidden_responses"], np.float32)[PERM]
    hb = np.asarray(inputs["hidden_biases"], np.float32)[PERM]
    orr = np.asarray(inputs["output_responses"], np.float32)
    ob = np.asarray(inputs["output_biases"], np.float32)

    wih_s = WS * (hr[:, None] * np.asarray(inputs["input_to_hidden"], np.float32)[PERM]).T
    whh_s = WS * (hr[:, None] *
                  np.asarray(inputs["hidden_to_hidden"], np.float32)[PERM][:, PERM]).T
    who_s = WS * (orr[:, None] *
                  np.asarray(inputs["hidden_to_output"], np.float32)[:, PERM]).T
    wio_s = WS * (orr[:, None] * np.asarray(inputs["input_to_output"], np.float32)).T

    def pack(w, ktiles):     # (ktiles*128, C) -> (128, ktiles, C)
        c = w.shape[1]
        return np.ascontiguousarray(
            w.reshape(ktiles, 128, c).transpose(1, 0, 2))

    # who: [p, kk, c] = who_s[kk*128 + p, c]
    who_p = pack(who_s, KH)
    # whh: [J*128 + p, j, col] = whh_s[(4J+j)*128 + p, col]
    whh_p = np.ascontiguousarray(
        whh_s.reshape(4, 4, 128, NH).transpose(0, 2, 1, 3).reshape(
            4 * 128, 4, NH))

    shared = {
        "wih": _q8(pack(wih_s, KI)),
        "whh": _q8(whh_p),
        "who": _q8(who_p),
        "wio": _q8(pack(wio_s, KI)),
        "hbc": np.ascontiguousarray(hb.reshape(KH, 128).T),
        "obc": np.ascontiguousarray(ob.reshape(KO, 128).T),
        "bmask": _make_bmask(),
    }
    in_maps = []
    for c in range(N_CORES):
        m = dict(shared)
        m["xT"] = _q8(pack(np.ascontiguousarray(x[c * BL:(c + 1) * BL].T), KI))
        in_maps.append(m)
    return in_maps


def _run(inputs, trace=False, tmpdir=None):
    nc = _get_nc()
    in_maps = _prep_in_maps(inputs)
    res = run_bass_kernel_spmd(nc, in_maps, core_ids=list(range(N_CORES)),
                               trace=trace, tmpdir=tmpdir)
    out = np.empty((B, NO), np.float32)
    for c in range(N_CORES):
        out[c * BL:(c + 1) * BL] = res.results[c]["outT"].T.astype(np.float32)
    return out, res


def kernel(**inputs) -> np.ndarray:
    out, _ = _run(inputs, trace=False)
    return out


if __name__ == "__main__":
    rng = np.random.default_rng(0)
    ins = {
        "inputs": rng.standard_normal((B, NI), dtype=np.float32),
        "input_to_hidden": rng.standard_normal((NH, NI), dtype=np.float32) * 0.02,
        "hidden_to_hidden": rng.standard_normal((NH, NH), dtype=np.float32) * 0.02,
        "output_to_hidden": rng.standard_normal((NH, NO), dtype=np.float32) * 0.02,
        "input_to_output": rng.standard_normal((NO, NI), dtype=np.float32) * 0.02,
        "hidden_to_output": rng.standard_normal((NO, NH), dtype=np.float32) * 0.02,
        "output_to_output": rng.standard_normal((NO, NO), dtype=np.float32) * 0.02,
        "hidden_responses": rng.standard_normal(NH, dtype=np.float32) * 0.1 + 1.0,
        "hidden_biases": rng.standard_normal(NH, dtype=np.float32) * 0.1,
        "output_responses": rng.standard_normal(NO, dtype=np.float32) * 0.1 + 1.0,
        "output_biases": rng.standard_normal(NO, dtype=np.float32) * 0.1,
    }
    out = kernel(**ins)
    print("kernel output", out.shape, out.dtype, out[:2, :4])


# revision 24
# speedup vs baseline: 1.0229x; 1.0077x over previous
"""Trainium2 Bass kernel for a 4-step differentiable recurrent net forward pass.

Reference computation (B=8192, NI=512, NH=2048, NO=512, 4 steps):
    activs = 0; outputs = 0
    repeat 4x:  pre = hr * (x @ Wih.T + activs @ Whh.T + outputs @ Woh.T) + hb
                activs = per_neuron_act(pre)        # tanh/sigmoid/relu by i%3
    out = sigmoid(or * (x @ Wio.T + outputs @ Woo.T + activs @ Who.T) + ob)

`outputs` is never written inside the loop, so the Woh/Woo terms vanish and
the x-projection P = hr*(x@Wih.T)+hb is loop-invariant (computed once).

Strategy: data-parallel on batch across 8 cores (1024 rows each). On-core
everything is feature-major (features on SBUF partitions, batch on the free
axis), so each matmul is W_tile.T @ X^T with stationary weights. All matmuls
run in fp8 e4m3 with DoubleRow perf mode (two k-tiles per instruction, 2x
MAC throughput). Weights are scaled by S=256 host-side so their ~0.02-scale
values sit in e4m3's normal range; the 1/S is folded into the activation
instruction's input scale. Activations are quantized to e4m3 unscaled (they
are O(1)). PSUM accumulates in fp32 throughout, so only operand quantization
loses precision (~1.3e-2 rel err on the final sigmoid outputs).
Host-side prep: hidden neurons are permuted so the three activation groups
are contiguous, hr/or are folded into the weight matrices, weights are
packed so each loads as one large contiguous DMA, and hb/ob are applied as
per-partition bias APs.
"""

import os

import numpy as np
import ml_dtypes

import concourse.bass as bass
import concourse.tile as tile
from concourse import bacc, mybir
from concourse.bass_utils import run_bass_kernel_spmd

B, NI, NH, NO = 8192, 512, 2048, 512
N_STEPS = 4
N_CORES = 8
BL = B // N_CORES          # batch rows per core
CH = 512                   # batch chunk (one PSUM bank of fp32)
NCH = BL // CH             # 2 chunks per core
KI = NI // 128             # 4 k-tiles over inputs
KH = NH // 128             # 16 k/m-tiles over hidden
KO = NO // 128             # 4 m-tiles over outputs

FP8 = mybir.dt.float8e4
BF16 = mybir.dt.bfloat16
F32 = mybir.dt.float32
AF = mybir.ActivationFunctionType
DR = mybir.MatmulPerfMode.DoubleRow
E4 = ml_dtypes.float8_e4m3

WS = 256.0                 # weight scale into fp8 range
IWS = 1.0 / WS             # folded back out at activation time

# hidden neurons regrouped as [all tanh | all sigmoid | all relu]
_idx = np.arange(NH)
PERM = np.concatenate([_idx[_idx % 3 == 0], _idx[_idx % 3 == 1], _idx[_idx % 3 == 2]])
_B1 = int((_idx % 3 == 0).sum())           # 683
_B2 = _B1 + int((_idx % 3 == 1).sum())     # 1366

# per m-tile: the single activation function, or None for the two mixed tiles
_TILE_FUNC = []
for _m in range(KH):
    _lo, _hi = _m * 128, (_m + 1) * 128
    _fs = set()
    for _f, _a, _b in ((AF.Tanh, 0, _B1), (AF.Sigmoid, _B1, _B2), (AF.Relu, _B2, NH)):
        if max(_lo, _a) < min(_hi, _b):
            _fs.add(_f)
    _TILE_FUNC.append(_fs.pop() if len(_fs) == 1 else None)

# mixed tiles: (major_func applied everywhere, minor_func, mask column block)
# partition sub-ranges must be 32-aligned on TRN2, so the minority strip is
# fixed up with a full-tile ACT + copy_predicated against a {0,1} mask
_BOUNDARY = {
    _B1 // 128: (AF.Sigmoid, AF.Tanh, 0),    # tile 5: parts < 43 are tanh
    _B2 // 128: (AF.Sigmoid, AF.Relu, 1),    # tile 10: parts >= 86 are relu
}


def _emit_hidden_act(nc, ps, blk, a_new, tmp_pool, bmask_t, hbc_t):
    """Run a 4-m-tile block of WS-scaled pre-activations through the grouped
    activations into a_new, applying the raw hidden bias inside the ACT.

    ps:    AP (128, 4*CH) holding m-tiles blk*4..blk*4+3 side by side
    a_new: SBUF tile (128, KH, CH) fp8, m-tile m lives at [:, m, :]
    hbc_t: (128, KH) f32 per-partition raw biases, column m for m-tile m
    """
    for mloc in range(4):
        m = blk * 4 + mloc
        bias = hbc_t[:, m:m + 1]
        src = ps[:, mloc * CH:(mloc + 1) * CH]
        if m in _BOUNDARY:
            major, minor, mb = _BOUNDARY[m]
            nc.scalar.activation(a_new[:, m:m + 1, :], src, major,
                                 bias=bias, scale=IWS)
            t = tmp_pool.tile([128, CH], FP8, tag="btmp", bufs=2, name="btmp")
            nc.scalar.activation(t[:], src, minor, bias=bias, scale=IWS)
            nc.vector.copy_predicated(
                a_new[:, m:m + 1, :],
                bmask_t[:, mb * CH:(mb + 1) * CH], t[:])
        else:
            nc.scalar.activation(a_new[:, m:m + 1, :], src, _TILE_FUNC[m],
                                 bias=bias, scale=IWS)


def _build_nc():
    nc = bacc.Bacc("TRN2", target_bir_lowering=False, debug=False,
                   num_devices=N_CORES, dynamic_dma_scratch_size=2048)

    # all operands host-packed so each loads as one large contiguous DMA;
    # k-tile k of a weight lives at [:, k, :] so DoubleRow k-pairs are
    # adjacent in the middle dim
    xT = nc.dram_tensor("xT", [128, KI, BL], FP8, kind="ExternalInput").ap()
    wih = nc.dram_tensor("wih", [128, KI, NH], FP8, kind="ExternalInput").ap()
    whh = nc.dram_tensor("whh", [4 * 128, 4, NH], FP8,
                         kind="ExternalInput").ap()
    who = nc.dram_tensor("who", [128, KH, NO], FP8, kind="ExternalInput").ap()
    wio = nc.dram_tensor("wio", [128, KI, NO], FP8, kind="ExternalInput").ap()
    hbc = nc.dram_tensor("hbc", [128, KH], F32, kind="ExternalInput").ap()
    obc = nc.dram_tensor("obc", [128, KO], F32, kind="ExternalInput").ap()
    bmask = nc.dram_tensor("bmask", [128, 2 * CH], mybir.dt.uint8,
                           kind="ExternalInput").ap()
    outT = nc.dram_tensor("outT", [NO, BL], BF16, kind="ExternalOutput").ap()

    with tile.TileContext(nc) as tc:
        with tc.tile_pool(name="w", bufs=1) as wpool, \
             tc.tile_pool(name="act", bufs=1) as apool, \
             tc.tile_pool(name="ps", bufs=2, space="PSUM") as pspool, \
             tc.tile_pool(name="out", bufs=4) as opool:

            # ---- stage inputs across all three DMA trigger paths (SP +
            # ACT HWDGE queues, GpSimd software DGE), ordered first-needed-
            # first. The first x-proj PSUM group (m-tiles 0-3) only needs
            # wih cols 0:1024 of both k-pairs + the chunk-0 x columns, so
            # those small pieces lead the sync queue and the first matmul
            # can issue ~3us after queue start. ----
            wih_m = wpool.tile([128, KI, NH], FP8, tag="projA", name="wihm")
            x_m = wpool.tile([128, KI, BL], FP8, tag="x", name="xm")
            hbc_t = wpool.tile([128, KH], F32, tag="hbc")
            obc_t = wpool.tile([128, KO], F32, tag="obc")
            bmask_t = wpool.tile([128, 2 * CH], mybir.dt.uint8, tag="bmask")
            wio_m = wpool.tile([128, KI, NO], FP8, tag="wio", name="wiom")
            whh_m = [wpool.tile([128, 4, NH], FP8, tag=f"whhJ{J}",
                                name=f"whhJ{J}") for J in range(4)]
            # ---- PE warmup: ~3.5us of dummy matmuls on an uninitialized
            # tile (no DMA dependency) so the HAM clock-gate is at 8/8
            # before the first real matmul issues (~12us in, behind DMA) ----
            warm_t = wpool.tile([128, 2, CH], FP8, tag="warm", name="warm")
            nc.gpsimd.memset(warm_t[:], 0.0)
            ps_w = pspool.tile([128, 4 * CH], F32, tag="ps", name="psw")
            for _w in range(16):
                nc.tensor.matmul(
                    ps_w[:, (_w % 4) * CH:(_w % 4 + 1) * CH],
                    warm_t[:, :, 0:128], warm_t[:],
                    start=True, stop=True, perf_mode=DR,
                    skip_group_check=True)

            nc.sync.dma_start(wih_m[:, 0:2, :], wih[:, 0:2, :])
            nc.sync.dma_start(wih_m[:, 2:4, :], wih[:, 2:4, :])
            nc.scalar.dma_start(x_m[:, 0:2, :], xT[:, 0:2, :])
            nc.scalar.dma_start(hbc_t[:], hbc[:])
            nc.scalar.dma_start(x_m[:, 2:4, :], xT[:, 2:4, :])
            nc.scalar.dma_start(bmask_t[:], bmask[:])
            nc.gpsimd.dma_start(wio_m[:], wio[:])
            # whh J0-J2 land in m-column pieces, m-major, so the first hh
            # step's m-block b only waits on pieces 0..b and DMA pipelines
            # against the m-block progression; J3 rides the gpsimd software
            # DGE as one contiguous transfer (strided gpsimd DMAs complete
            # their semaphore unreliably)
            nc.gpsimd.dma_start(whh_m[3][:], whh[3 * 128:4 * 128])
            for mb in range(4):
                lo, hi = mb * 512, (mb + 1) * 512
                for J, eng in ((0, nc.sync), (1, nc.scalar), (2, nc.sync)):
                    eng.dma_start(whh_m[J][:, :, lo:hi],
                                  whh[J * 128:(J + 1) * 128, :, lo:hi])
            nc.scalar.dma_start(obc_t[:], obc[:])

            def whh_pair(kp):      # lhsT k-pair AP for hh k-tiles 2kp,2kp+1
                J, j = divmod(2 * kp, 4)
                return whh_m[J], j

            # ---- per-chunk x-projection P and first-step activations ----
            P = {}
            A = {}
            for c in range(NCH):
                P[c] = apool.tile([128, KH * CH], BF16, tag=f"P{c}",
                                  name=f"P{c}")
                a1 = apool.tile([128, KH, CH], FP8, tag="A", bufs=3,
                                name=f"A1c{c}")
                for blk in range(4):
                    ps = pspool.tile([128, 4 * CH], F32, tag="ps", name="psb")
                    for kp in range(KI // 2):
                        for mloc in range(4):
                            m = blk * 4 + mloc
                            nc.tensor.matmul(
                                ps[:, mloc * CH:(mloc + 1) * CH],
                                wih_m[:, 2 * kp:2 * kp + 2,
                                      m * 128:(m + 1) * 128],
                                x_m[:, 2 * kp:2 * kp + 2,
                                    c * CH:(c + 1) * CH],
                                start=(kp == 0), stop=(kp == KI // 2 - 1),
                                perf_mode=DR)
                    # P holds the raw WS-scaled x-projection (bias is applied
                    # inside the ACTs); a single copy frees the PSUM slot
                    nc.vector.tensor_copy(
                        P[c][:, blk * 4 * CH:(blk + 1) * 4 * CH], ps[:])
                    _emit_hidden_act(nc, P[c][:, blk * 4 * CH:(blk + 1) * 4 * CH],
                                     blk, a1, opool, bmask_t, hbc_t)
                A[c] = a1

            # ---- whh-independent output x-projection (fills the window
            # while the whh load is still in flight) ----
            outx = {}
            for c in range(NCH):
                outx[c] = apool.tile([128, KO * CH], BF16, tag=f"outx{c}",
                                     name=f"outx{c}")
                ps = pspool.tile([128, 4 * CH], F32, tag="ps", name="psb")
                for kp in range(KI // 2):
                    for mo in range(KO):
                        nc.tensor.matmul(
                            ps[:, mo * CH:(mo + 1) * CH],
                            wio_m[:, 2 * kp:2 * kp + 2,
                                  mo * 128:(mo + 1) * 128],
                            x_m[:, 2 * kp:2 * kp + 2, c * CH:(c + 1) * CH],
                            start=(kp == 0), stop=(kp == KI // 2 - 1),
                            perf_mode=DR)
                nc.vector.tensor_copy(outx[c][:], ps[:])

            # ---- recurrent steps 2..4 ----
            def hh_step(c, s):
                a_new = apool.tile([128, KH, CH], FP8, tag="A", bufs=3,
                                   name=f"A{s + 2}c{c}")
                for blk in range(4):
                    ps = pspool.tile([128, 4 * CH], F32, tag="ps", name="psb")
                    for kp in range(KH // 2):
                        wt, j = whh_pair(kp)
                        for mloc in range(4):
                            m = blk * 4 + mloc
                            nc.tensor.matmul(
                                ps[:, mloc * CH:(mloc + 1) * CH],
                                wt[:, j:j + 2, m * 128:(m + 1) * 128],
                                A[c][:, 2 * kp:2 * kp + 2, :],
                                start=(kp == 0), stop=(kp == KH // 2 - 1),
                                perf_mode=DR)
                    # pre = psum + P into an SBUF temp: a single PSUM read
                    # frees the bank; ACT then runs off SBUF
                    tmp = opool.tile([128, 4 * CH], F32, tag="pre", bufs=2,
                                     name="pre")
                    nc.vector.tensor_add(
                        tmp[:], ps[:], P[c][:, blk * 4 * CH:(blk + 1) * 4 * CH])
                    _emit_hidden_act(nc, tmp, blk, a_new, opool, bmask_t,
                                     hbc_t)
                A[c] = a_new

            for s in range(N_STEPS - 2):
                for c in range(NCH):
                    hh_step(c, s)
            hh_step(0, N_STEPS - 2)  # chunk 1's final step emitted after who

            # ---- output layer (who reuses the wih slot); chunk 0's
            # output overlaps chunk 1's final hh step ----
            who_m = wpool.tile([128, KH, NO], FP8, tag="projA", name="whom")
            nc.sync.dma_start(who_m[:], who[:])

            def out_chunk(c):
                for mo in range(KO):
                    pso = pspool.tile([128, CH], F32, tag="ps", name="pso")
                    oap = pso[:]
                    for kp in range(KH // 2):
                        nc.tensor.matmul(
                            oap,
                            who_m[:, 2 * kp:2 * kp + 2,
                                  mo * 128:(mo + 1) * 128],
                            A[c][:, 2 * kp:2 * kp + 2, :],
                            start=(kp == 0), stop=(kp == KH // 2 - 1),
                            perf_mode=DR)
                    to = opool.tile([128, CH], F32, tag="preo", bufs=2,
                                    name="preo")
                    nc.vector.tensor_add(
                        to[:], oap, outx[c][:, mo * CH:(mo + 1) * CH])
                    o = opool.tile([128, CH], BF16, tag="o", bufs=2, name="o")
                    nc.scalar.activation(o[:], to[:], AF.Sigmoid,
                                         bias=obc_t[:, mo:mo + 1], scale=IWS)
                    eng = nc.sync if mo % 2 == 0 else nc.scalar
                    eng.dma_start(
                        outT[mo * 128:(mo + 1) * 128, c * CH:(c + 1) * CH],
                        o[:])

            hh_step(1, N_STEPS - 2)
            out_chunk(0)
            out_chunk(1)

    nc.compile()
    return nc


_NC_CACHE = None


def _get_nc():
    global _NC_CACHE
    if _NC_CACHE is None:
        _NC_CACHE = _build_nc()
    return _NC_CACHE


def _make_bmask():
    m = np.zeros((128, 2 * CH), np.uint8)
    m[:_B1 - (_B1 // 128) * 128, 0:CH] = 1          # tile 5: parts < 43 tanh
    m[_B2 - (_B2 // 128) * 128:, CH:2 * CH] = 1     # tile 10: parts >= 86 relu
    return m


def _q8(a):
    return np.clip(a, -240.0, 240.0).astype(E4)


def _prep_in_maps(inputs):
    x = np.asarray(inputs["inputs"], np.float32)
    hr = np.asarray(inputs["hidden_responses"], np.float32)[PERM]
    hb = np.asarray(inputs["hidden_biases"], np.float32)[PERM]
    orr = np.asarray(inputs["output_responses"], np.float32)
    ob = np.asarray(inputs["output_biases"], np.float32)

    wih_s = WS * (hr[:, None] * np.asarray(inputs["input_to_hidden"], np.float32)[PERM]).T
    whh_s = WS * (hr[:, None] *
                  np.asarray(inputs["hidden_to_hidden"], np.float32)[PERM][:, PERM]).T
    who_s = WS * (orr[:, None] *
                  np.asarray(inputs["hidden_to_output"], np.float32)[:, PERM]).T
    wio_s = WS * (orr[:, None] * np.asarray(inputs["input_to_output"], np.float32)).T

    def pack(w, ktiles):     # (ktiles*128, C) -> (128, ktiles, C)
        c = w.shape[1]
        return np.ascontiguousarray(
            w.reshape(ktiles, 128, c).transpose(1, 0, 2))

    # who: [p, kk, c] = who_s[kk*128 + p, c]
    who_p = pack(who_s, KH)
    # whh: [J*128 + p, j, col] = whh_s[(4J+j)*128 + p, col]
    whh_p = np.ascontiguousarray(
        whh_s.reshape(4, 4, 128, NH).transpose(0, 2, 1, 3).reshape(
            4 * 128, 4, NH))

    shared = {
        "wih": _q8(pack(wih_s, KI)),
        "whh": _q8(whh_p),
        "who": _q8(who_p),
        "wio": _q8(pack(wio_s, KI)),
        "hbc": np.ascontiguousarray(hb.reshape(KH, 128).T),
        "obc": np.ascontiguousarray(ob.reshape(KO, 128).T),
        "bmask": _make_bmask(),
    }
    in_maps = []
    for c in range(N_CORES):
        m = dict(shared)
        m["xT"] = _q8(pack(np.ascontiguousarray(x[c * BL:(c + 1) * BL].T), KI))
        in_maps.append(m)
    return in_maps


def _run(inputs, trace=False, tmpdir=None):
    nc = _get_nc()
    in_maps = _prep_in_maps(inputs)
    res = run_bass_kernel_spmd(nc, in_maps, core_ids=list(range(N_CORES)),
                               trace=trace, tmpdir=tmpdir)
    out = np.empty((B, NO), np.float32)
    for c in range(N_CORES):
        out[c * BL:(c + 1) * BL] = res.results[c]["outT"].T.astype(np.float32)
    return out, res


def kernel(**inputs) -> np.ndarray:
    out, _ = _run(inputs, trace=False)
    return out


if __name__ == "__main__":
    rng = np.random.default_rng(0)
    ins = {
        "inputs": rng.standard_normal((B, NI), dtype=np.float32),
        "input_to_hidden": rng.standard_normal((NH, NI), dtype=np.float32) * 0.02,
        "hidden_to_hidden": rng.standard_normal((NH, NH), dtype=np.float32) * 0.02,
        "output_to_hidden": rng.standard_normal((NH, NO), dtype=np.float32) * 0.02,
        "input_to_output": rng.standard_normal((NO, NI), dtype=np.float32) * 0.02,
        "hidden_to_output": rng.standard_normal((NO, NH), dtype=np.float32) * 0.02,
        "output_to_output": rng.standard_normal((NO, NO), dtype=np.float32) * 0.02,
        "hidden_responses": rng.standard_normal(NH, dtype=np.float32) * 0.1 + 1.0,
        "hidden_biases": rng.standard_normal(NH, dtype=np.float32) * 0.1,
        "output_responses": rng.standard_normal(NO, dtype=np.float32) * 0.1 + 1.0,
        "output_biases": rng.standard_normal(NO, dtype=np.float32) * 0.1,
    }
    out = kernel(**ins)
    print("kernel output", out.shape, out.dtype, out[:2, :4])


# revision 25
# speedup vs baseline: 1.0233x; 1.0004x over previous
"""Trainium2 Bass kernel for a 4-step differentiable recurrent net forward pass.

Reference computation (B=8192, NI=512, NH=2048, NO=512, 4 steps):
    activs = 0; outputs = 0
    repeat 4x:  pre = hr * (x @ Wih.T + activs @ Whh.T + outputs @ Woh.T) + hb
                activs = per_neuron_act(pre)        # tanh/sigmoid/relu by i%3
    out = sigmoid(or * (x @ Wio.T + outputs @ Woo.T + activs @ Who.T) + ob)

`outputs` is never written inside the loop, so the Woh/Woo terms vanish and
the x-projection P = hr*(x@Wih.T)+hb is loop-invariant (computed once).

Strategy: data-parallel on batch across 8 cores (1024 rows each). On-core
everything is feature-major (features on SBUF partitions, batch on the free
axis), so each matmul is W_tile.T @ X^T with stationary weights. All matmuls
run in fp8 e4m3 with DoubleRow perf mode (two k-tiles per instruction, 2x
MAC throughput). Weights are scaled by S=256 host-side so their ~0.02-scale
values sit in e4m3's normal range; the 1/S is folded into the activation
instruction's input scale. Activations are quantized to e4m3 unscaled (they
are O(1)). PSUM accumulates in fp32 throughout, so only operand quantization
loses precision (~1.3e-2 rel err on the final sigmoid outputs).
Host-side prep: hidden neurons are permuted so the three activation groups
are contiguous, hr/or are folded into the weight matrices, weights are
packed so each loads as one large contiguous DMA, and hb/ob are applied as
per-partition bias APs.
"""

import os

import numpy as np
import ml_dtypes

import concourse.bass as bass
import concourse.tile as tile
from concourse import bacc, mybir
from concourse.bass_utils import run_bass_kernel_spmd

B, NI, NH, NO = 8192, 512, 2048, 512
N_STEPS = 4
N_CORES = 8
BL = B // N_CORES          # batch rows per core
CH = 512                   # batch chunk (one PSUM bank of fp32)
NCH = BL // CH             # 2 chunks per core
KI = NI // 128             # 4 k-tiles over inputs
KH = NH // 128             # 16 k/m-tiles over hidden
KO = NO // 128             # 4 m-tiles over outputs

FP8 = mybir.dt.float8e4
BF16 = mybir.dt.bfloat16
F32 = mybir.dt.float32
AF = mybir.ActivationFunctionType
DR = mybir.MatmulPerfMode.DoubleRow
E4 = ml_dtypes.float8_e4m3

WS = 256.0                 # weight scale into fp8 range
IWS = 1.0 / WS             # folded back out at activation time

# hidden neurons regrouped as [all tanh | all sigmoid | all relu]
_idx = np.arange(NH)
PERM = np.concatenate([_idx[_idx % 3 == 0], _idx[_idx % 3 == 1], _idx[_idx % 3 == 2]])
_B1 = int((_idx % 3 == 0).sum())           # 683
_B2 = _B1 + int((_idx % 3 == 1).sum())     # 1366

# per m-tile: the single activation function, or None for the two mixed tiles
_TILE_FUNC = []
for _m in range(KH):
    _lo, _hi = _m * 128, (_m + 1) * 128
    _fs = set()
    for _f, _a, _b in ((AF.Tanh, 0, _B1), (AF.Sigmoid, _B1, _B2), (AF.Relu, _B2, NH)):
        if max(_lo, _a) < min(_hi, _b):
            _fs.add(_f)
    _TILE_FUNC.append(_fs.pop() if len(_fs) == 1 else None)

# mixed tiles: (major_func applied everywhere, minor_func, mask column block)
# partition sub-ranges must be 32-aligned on TRN2, so the minority strip is
# fixed up with a full-tile ACT + copy_predicated against a {0,1} mask
_BOUNDARY = {
    _B1 // 128: (AF.Sigmoid, AF.Tanh, 0),    # tile 5: parts < 43 are tanh
    _B2 // 128: (AF.Sigmoid, AF.Relu, 1),    # tile 10: parts >= 86 are relu
}


def _emit_hidden_act(nc, ps, blk, a_new, tmp_pool, bmask_t, hbc_t):
    """Run a 4-m-tile block of WS-scaled pre-activations through the grouped
    activations into a_new, applying the raw hidden bias inside the ACT.

    ps:    AP (128, 4*CH) holding m-tiles blk*4..blk*4+3 side by side
    a_new: SBUF tile (128, KH, CH) fp8, m-tile m lives at [:, m, :]
    hbc_t: (128, KH) f32 per-partition raw biases, column m for m-tile m
    """
    for mloc in range(4):
        m = blk * 4 + mloc
        bias = hbc_t[:, m:m + 1]
        src = ps[:, mloc * CH:(mloc + 1) * CH]
        if m in _BOUNDARY:
            major, minor, mb = _BOUNDARY[m]
            nc.scalar.activation(a_new[:, m:m + 1, :], src, major,
                                 bias=bias, scale=IWS)
            t = tmp_pool.tile([128, CH], FP8, tag="btmp", bufs=2, name="btmp")
            nc.scalar.activation(t[:], src, minor, bias=bias, scale=IWS)
            nc.vector.copy_predicated(
                a_new[:, m:m + 1, :],
                bmask_t[:, mb * CH:(mb + 1) * CH], t[:])
        else:
            nc.scalar.activation(a_new[:, m:m + 1, :], src, _TILE_FUNC[m],
                                 bias=bias, scale=IWS)


def _build_nc():
    nc = bacc.Bacc("TRN2", target_bir_lowering=False, debug=False,
                   num_devices=N_CORES, dynamic_dma_scratch_size=2048)

    # all operands host-packed so each loads as one large contiguous DMA;
    # k-tile k of a weight lives at [:, k, :] so DoubleRow k-pairs are
    # adjacent in the middle dim
    xT = nc.dram_tensor("xT", [128, KI, BL], FP8, kind="ExternalInput").ap()
    wih = nc.dram_tensor("wih", [128, KI, NH], FP8, kind="ExternalInput").ap()
    whh = nc.dram_tensor("whh", [4 * 128, 4, NH], FP8,
                         kind="ExternalInput").ap()
    who = nc.dram_tensor("who", [128, KH, NO], FP8, kind="ExternalInput").ap()
    wio = nc.dram_tensor("wio", [128, KI, NO], FP8, kind="ExternalInput").ap()
    hbc = nc.dram_tensor("hbc", [128, KH], F32, kind="ExternalInput").ap()
    obc = nc.dram_tensor("obc", [128, KO], F32, kind="ExternalInput").ap()
    bmask = nc.dram_tensor("bmask", [128, 2 * CH], mybir.dt.uint8,
                           kind="ExternalInput").ap()
    outT = nc.dram_tensor("outT", [NO, BL], BF16, kind="ExternalOutput").ap()

    with tile.TileContext(nc) as tc:
        with tc.tile_pool(name="w", bufs=1) as wpool, \
             tc.tile_pool(name="act", bufs=1) as apool, \
             tc.tile_pool(name="ps", bufs=2, space="PSUM") as pspool, \
             tc.tile_pool(name="out", bufs=4) as opool:

            # ---- stage inputs across all three DMA trigger paths (SP +
            # ACT HWDGE queues, GpSimd software DGE), ordered first-needed-
            # first. The first x-proj PSUM group (m-tiles 0-3) only needs
            # wih cols 0:1024 of both k-pairs + the chunk-0 x columns, so
            # those small pieces lead the sync queue and the first matmul
            # can issue ~3us after queue start. ----
            wih_m = wpool.tile([128, KI, NH], FP8, tag="projA", name="wihm")
            x_m = wpool.tile([128, KI, BL], FP8, tag="x", name="xm")
            hbc_t = wpool.tile([128, KH], F32, tag="hbc")
            obc_t = wpool.tile([128, KO], F32, tag="obc")
            bmask_t = wpool.tile([128, 2 * CH], mybir.dt.uint8, tag="bmask")
            wio_m = wpool.tile([128, KI, NO], FP8, tag="wio", name="wiom")
            whh_m = [wpool.tile([128, 4, NH], FP8, tag=f"whhJ{J}",
                                name=f"whhJ{J}") for J in range(4)]
            # ---- PE warmup: ~3.5us of dummy matmuls on an uninitialized
            # tile (no DMA dependency) so the HAM clock-gate is at 8/8
            # before the first real matmul issues (~12us in, behind DMA) ----
            warm_t = wpool.tile([128, 2, CH], FP8, tag="warm", name="warm")
            nc.gpsimd.memset(warm_t[:], 0.0)
            ps_w = pspool.tile([128, 4 * CH], F32, tag="ps", name="psw")
            for _w in range(16):
                nc.tensor.matmul(
                    ps_w[:, (_w % 4) * CH:(_w % 4 + 1) * CH],
                    warm_t[:, :, 0:128], warm_t[:],
                    start=True, stop=True, perf_mode=DR,
                    skip_group_check=True)

            # wih lands in m-column halves matching x-proj block order
            # (blocks 0-1 need cols 0:1024 of both k-pairs first); x lands
            # chunk-0 columns first
            HH2 = NH // 2
            nc.sync.dma_start(wih_m[:, 0:2, 0:HH2], wih[:, 0:2, 0:HH2])
            nc.sync.dma_start(wih_m[:, 2:4, 0:HH2], wih[:, 2:4, 0:HH2])
            nc.sync.dma_start(wih_m[:, 0:2, HH2:NH], wih[:, 0:2, HH2:NH])
            nc.sync.dma_start(wih_m[:, 2:4, HH2:NH], wih[:, 2:4, HH2:NH])
            nc.scalar.dma_start(x_m[:, 0:2, 0:CH], xT[:, 0:2, 0:CH])
            nc.scalar.dma_start(x_m[:, 2:4, 0:CH], xT[:, 2:4, 0:CH])
            nc.scalar.dma_start(hbc_t[:], hbc[:])
            nc.scalar.dma_start(x_m[:, 0:2, CH:BL], xT[:, 0:2, CH:BL])
            nc.scalar.dma_start(x_m[:, 2:4, CH:BL], xT[:, 2:4, CH:BL])
            nc.scalar.dma_start(bmask_t[:], bmask[:])
            nc.gpsimd.dma_start(wio_m[:], wio[:])
            # whh J0-J2 land in m-column pieces, m-major, so the first hh
            # step's m-block b only waits on pieces 0..b and DMA pipelines
            # against the m-block progression; J3 rides the gpsimd software
            # DGE as one contiguous transfer (strided gpsimd DMAs complete
            # their semaphore unreliably)
            nc.gpsimd.dma_start(whh_m[3][:], whh[3 * 128:4 * 128])
            for mb in range(4):
                lo, hi = mb * 512, (mb + 1) * 512
                for J, eng in ((0, nc.sync), (1, nc.scalar), (2, nc.sync)):
                    eng.dma_start(whh_m[J][:, :, lo:hi],
                                  whh[J * 128:(J + 1) * 128, :, lo:hi])
            nc.scalar.dma_start(obc_t[:], obc[:])

            def whh_pair(kp):      # lhsT k-pair AP for hh k-tiles 2kp,2kp+1
                J, j = divmod(2 * kp, 4)
                return whh_m[J], j

            # ---- per-chunk x-projection P and first-step activations ----
            P = {}
            A = {}
            for c in range(NCH):
                P[c] = apool.tile([128, KH * CH], BF16, tag=f"P{c}",
                                  name=f"P{c}")
                a1 = apool.tile([128, KH, CH], FP8, tag="A", bufs=3,
                                name=f"A1c{c}")
                for blk in range(4):
                    ps = pspool.tile([128, 4 * CH], F32, tag="ps", name="psb")
                    for kp in range(KI // 2):
                        for mloc in range(4):
                            m = blk * 4 + mloc
                            nc.tensor.matmul(
                                ps[:, mloc * CH:(mloc + 1) * CH],
                                wih_m[:, 2 * kp:2 * kp + 2,
                                      m * 128:(m + 1) * 128],
                                x_m[:, 2 * kp:2 * kp + 2,
                                    c * CH:(c + 1) * CH],
                                start=(kp == 0), stop=(kp == KI // 2 - 1),
                                perf_mode=DR)
                    # P holds the raw WS-scaled x-projection (bias is applied
                    # inside the ACTs); a single copy frees the PSUM slot
                    nc.vector.tensor_copy(
                        P[c][:, blk * 4 * CH:(blk + 1) * 4 * CH], ps[:])
                    _emit_hidden_act(nc, P[c][:, blk * 4 * CH:(blk + 1) * 4 * CH],
                                     blk, a1, opool, bmask_t, hbc_t)
                A[c] = a1

            # ---- whh-independent output x-projection (fills the window
            # while the whh load is still in flight) ----
            outx = {}
            for c in range(NCH):
                outx[c] = apool.tile([128, KO * CH], BF16, tag=f"outx{c}",
                                     name=f"outx{c}")
                ps = pspool.tile([128, 4 * CH], F32, tag="ps", name="psb")
                for kp in range(KI // 2):
                    for mo in range(KO):
                        nc.tensor.matmul(
                            ps[:, mo * CH:(mo + 1) * CH],
                            wio_m[:, 2 * kp:2 * kp + 2,
                                  mo * 128:(mo + 1) * 128],
                            x_m[:, 2 * kp:2 * kp + 2, c * CH:(c + 1) * CH],
                            start=(kp == 0), stop=(kp == KI // 2 - 1),
                            perf_mode=DR)
                nc.vector.tensor_copy(outx[c][:], ps[:])

            # ---- recurrent steps 2..4 ----
            def hh_step(c, s):
                a_new = apool.tile([128, KH, CH], FP8, tag="A", bufs=3,
                                   name=f"A{s + 2}c{c}")
                for blk in range(4):
                    ps = pspool.tile([128, 4 * CH], F32, tag="ps", name="psb")
                    for kp in range(KH // 2):
                        wt, j = whh_pair(kp)
                        for mloc in range(4):
                            m = blk * 4 + mloc
                            nc.tensor.matmul(
                                ps[:, mloc * CH:(mloc + 1) * CH],
                                wt[:, j:j + 2, m * 128:(m + 1) * 128],
                                A[c][:, 2 * kp:2 * kp + 2, :],
                                start=(kp == 0), stop=(kp == KH // 2 - 1),
                                perf_mode=DR)
                    # pre = psum + P into an SBUF temp: a single PSUM read
                    # frees the bank; ACT then runs off SBUF
                    tmp = opool.tile([128, 4 * CH], F32, tag="pre", bufs=2,
                                     name="pre")
                    nc.vector.tensor_add(
                        tmp[:], ps[:], P[c][:, blk * 4 * CH:(blk + 1) * 4 * CH])
                    _emit_hidden_act(nc, tmp, blk, a_new, opool, bmask_t,
                                     hbc_t)
                A[c] = a_new

            for s in range(N_STEPS - 2):
                for c in range(NCH):
                    hh_step(c, s)
            hh_step(0, N_STEPS - 2)  # chunk 1's final step emitted after who

            # ---- output layer (who reuses the wih slot); chunk 0's
            # output overlaps chunk 1's final hh step ----
            who_m = wpool.tile([128, KH, NO], FP8, tag="projA", name="whom")
            nc.sync.dma_start(who_m[:], who[:])

            def out_chunk(c):
                for mo in range(KO):
                    pso = pspool.tile([128, CH], F32, tag="ps", name="pso")
                    oap = pso[:]
                    for kp in range(KH // 2):
                        nc.tensor.matmul(
                            oap,
                            who_m[:, 2 * kp:2 * kp + 2,
                                  mo * 128:(mo + 1) * 128],
                            A[c][:, 2 * kp:2 * kp + 2, :],
                            start=(kp == 0), stop=(kp == KH // 2 - 1),
                            perf_mode=DR)
                    to = opool.tile([128, CH], F32, tag="preo", bufs=2,
                                    name="preo")
                    nc.vector.tensor_add(
                        to[:], oap, outx[c][:, mo * CH:(mo + 1) * CH])
                    o = opool.tile([128, CH], BF16, tag="o", bufs=2, name="o")
                    nc.scalar.activation(o[:], to[:], AF.Sigmoid,
                                         bias=obc_t[:, mo:mo + 1], scale=IWS)
                    eng = nc.sync if mo % 2 == 0 else nc.scalar
                    eng.dma_start(
                        outT[mo * 128:(mo + 1) * 128, c * CH:(c + 1) * CH],
                        o[:])

            hh_step(1, N_STEPS - 2)
            out_chunk(0)
            out_chunk(1)

    nc.compile()
    return nc


_NC_CACHE = None


def _get_nc():
    global _NC_CACHE
    if _NC_CACHE is None:
        _NC_CACHE = _build_nc()
    return _NC_CACHE


def _make_bmask():
    m = np.zeros((128, 2 * CH), np.uint8)
    m[:_B1 - (_B1 // 128) * 128, 0:CH] = 1          # tile 5: parts < 43 tanh
    m[_B2 - (_B2 // 128) * 128:, CH:2 * CH] = 1     # tile 10: parts >= 86 relu
    return m


def _q8(a):
    return np.clip(a, -240.0, 240.0).astype(E4)


def _prep_in_maps(inputs):
    x = np.asarray(inputs["inputs"], np.float32)
    hr = np.asarray(inputs["hidden_responses"], np.float32)[PERM]
    hb = np.asarray(inputs["hidden_biases"], np.float32)[PERM]
    orr = np.asarray(inputs["output_responses"], np.float32)
    ob = np.asarray(inputs["output_biases"], np.float32)

    wih_s = WS * (hr[:, None] * np.asarray(inputs["input_to_hidden"], np.float32)[PERM]).T
    whh_s = WS * (hr[:, None] *
                  np.asarray(inputs["hidden_to_hidden"], np.float32)[PERM][:, PERM]).T
    who_s = WS * (orr[:, None] *
                  np.asarray(inputs["hidden_to_output"], np.float32)[:, PERM]).T
    wio_s = WS * (orr[:, None] * np.asarray(inputs["input_to_output"], np.float32)).T

    def pack(w, ktiles):     # (ktiles*128, C) -> (128, ktiles, C)
        c = w.shape[1]
        return np.ascontiguousarray(
            w.reshape(ktiles, 128, c).transpose(1, 0, 2))

    # who: [p, kk, c] = who_s[kk*128 + p, c]
    who_p = pack(who_s, KH)
    # whh: [J*128 + p, j, col] = whh_s[(4J+j)*128 + p, col]
    whh_p = np.ascontiguousarray(
        whh_s.reshape(4, 4, 128, NH).transpose(0, 2, 1, 3).reshape(
            4 * 128, 4, NH))

    shared = {
        "wih": _q8(pack(wih_s, KI)),
        "whh": _q8(whh_p),
        "who": _q8(who_p),
        "wio": _q8(pack(wio_s, KI)),
        "hbc": np.ascontiguousarray(hb.reshape(KH, 128).T),
        "obc": np.ascontiguousarray(ob.reshape(KO, 128).T),
        "bmask": _make_bmask(),
    }
    in_maps = []
    for c in range(N_CORES):
        m = dict(shared)
        m["xT"] = _q8(pack(np.ascontiguousarray(x[c * BL:(c + 1) * BL].T), KI))
        in_maps.append(m)
    return in_maps


def _run(inputs, trace=False, tmpdir=None):
    nc = _get_nc()
    in_maps = _prep_in_maps(inputs)
    res = run_bass_kernel_spmd(nc, in_maps, core_ids=list(range(N_CORES)),
                               trace=trace, tmpdir=tmpdir)
    out = np.empty((B, NO), np.float32)
    for c in range(N_CORES):
        out[c * BL:(c + 1) * BL] = res.results[c]["outT"].T.astype(np.float32)
    return out, res


def kernel(**inputs) -> np.ndarray:
    out, _ = _run(inputs, trace=False)
    return out


if __name__ == "__main__":
    rng = np.random.default_rng(0)
    ins = {
        "inputs": rng.standard_normal((B, NI), dtype=np.float32),
        "input_to_hidden": rng.standard_normal((NH, NI), dtype=np.float32) * 0.02,
        "hidden_to_hidden": rng.standard_normal((NH, NH), dtype=np.float32) * 0.02,
        "output_to_hidden": rng.standard_normal((NH, NO), dtype=np.float32) * 0.02,
        "input_to_output": rng.standard_normal((NO, NI), dtype=np.float32) * 0.02,
        "hidden_to_output": rng.standard_normal((NO, NH), dtype=np.float32) * 0.02,
        "output_to_output": rng.standard_normal((NO, NO), dtype=np.float32) * 0.02,
        "hidden_responses": rng.standard_normal(NH, dtype=np.float32) * 0.1 + 1.0,
        "hidden_biases": rng.standard_normal(NH, dtype=np.float32) * 0.1,
        "output_responses": rng.standard_normal(NO, dtype=np.float32) * 0.1 + 1.0,
        "output_biases": rng.standard_normal(NO, dtype=np.float32) * 0.1,
    }
    out = kernel(**ins)
    print("kernel output", out.shape, out.dtype, out[:2, :4])


# revision 26
# speedup vs baseline: 1.0234x; 1.0001x over previous
"""Trainium2 Bass kernel for a 4-step differentiable recurrent net forward pass.

Reference computation (B=8192, NI=512, NH=2048, NO=512, 4 steps):
    activs = 0; outputs = 0
    repeat 4x:  pre = hr * (x @ Wih.T + activs @ Whh.T + outputs @ Woh.T) + hb
                activs = per_neuron_act(pre)        # tanh/sigmoid/relu by i%3
    out = sigmoid(or * (x @ Wio.T + outputs @ Woo.T + activs @ Who.T) + ob)

`outputs` is never written inside the loop, so the Woh/Woo terms vanish and
the x-projection P = hr*(x@Wih.T)+hb is loop-invariant (computed once).

Strategy: data-parallel on batch across 8 cores (1024 rows each). On-core
everything is feature-major (features on SBUF partitions, batch on the free
axis), so each matmul is W_tile.T @ X^T with stationary weights. All matmuls
run in fp8 e4m3 with DoubleRow perf mode (two k-tiles per instruction, 2x
MAC throughput). Weights are scaled by S=256 host-side so their ~0.02-scale
values sit in e4m3's normal range; the 1/S is folded into the activation
instruction's input scale. Activations are quantized to e4m3 unscaled (they
are O(1)). PSUM accumulates in fp32 throughout, so only operand quantization
loses precision (~1.3e-2 rel err on the final sigmoid outputs).
Host-side prep: hidden neurons are permuted so the three activation groups
are contiguous, hr/or are folded into the weight matrices, weights are
packed so each loads as one large contiguous DMA, and hb/ob are applied as
per-partition bias APs.
"""

import os

import numpy as np
import ml_dtypes

import concourse.bass as bass
import concourse.tile as tile
from concourse import bacc, mybir
from concourse.bass_utils import run_bass_kernel_spmd

B, NI, NH, NO = 8192, 512, 2048, 512
N_STEPS = 4
N_CORES = 8
BL = B // N_CORES          # batch rows per core
CH = 512                   # batch chunk (one PSUM bank of fp32)
NCH = BL // CH             # 2 chunks per core
KI = NI // 128             # 4 k-tiles over inputs
KH = NH // 128             # 16 k/m-tiles over hidden
KO = NO // 128             # 4 m-tiles over outputs

FP8 = mybir.dt.float8e4
BF16 = mybir.dt.bfloat16
F32 = mybir.dt.float32
AF = mybir.ActivationFunctionType
DR = mybir.MatmulPerfMode.DoubleRow
E4 = ml_dtypes.float8_e4m3

WS = 256.0                 # weight scale into fp8 range
IWS = 1.0 / WS             # folded back out at activation time

# hidden neurons regrouped as [all tanh | all sigmoid | all relu]
_idx = np.arange(NH)
PERM = np.concatenate([_idx[_idx % 3 == 0], _idx[_idx % 3 == 1], _idx[_idx % 3 == 2]])
_B1 = int((_idx % 3 == 0).sum())           # 683
_B2 = _B1 + int((_idx % 3 == 1).sum())     # 1366

# per m-tile: the single activation function, or None for the two mixed tiles
_TILE_FUNC = []
for _m in range(KH):
    _lo, _hi = _m * 128, (_m + 1) * 128
    _fs = set()
    for _f, _a, _b in ((AF.Tanh, 0, _B1), (AF.Sigmoid, _B1, _B2), (AF.Relu, _B2, NH)):
        if max(_lo, _a) < min(_hi, _b):
            _fs.add(_f)
    _TILE_FUNC.append(_fs.pop() if len(_fs) == 1 else None)

# mixed tiles: (major_func applied everywhere, minor_func, mask column block)
# partition sub-ranges must be 32-aligned on TRN2, so the minority strip is
# fixed up with a full-tile ACT + copy_predicated against a {0,1} mask
_BOUNDARY = {
    _B1 // 128: (AF.Sigmoid, AF.Tanh, 0),    # tile 5: parts < 43 are tanh
    _B2 // 128: (AF.Sigmoid, AF.Relu, 1),    # tile 10: parts >= 86 are relu
}


def _emit_hidden_act(nc, ps, blk, a_new, tmp_pool, bmask_t, hbc_t):
    """Run a 4-m-tile block of WS-scaled pre-activations through the grouped
    activations into a_new, applying the raw hidden bias inside the ACT.

    ps:    AP (128, 4*CH) holding m-tiles blk*4..blk*4+3 side by side
    a_new: SBUF tile (128, KH, CH) fp8, m-tile m lives at [:, m, :]
    hbc_t: (128, KH) f32 per-partition raw biases, column m for m-tile m
    """
    for mloc in range(4):
        m = blk * 4 + mloc
        bias = hbc_t[:, m:m + 1]
        src = ps[:, mloc * CH:(mloc + 1) * CH]
        if m in _BOUNDARY:
            major, minor, mb = _BOUNDARY[m]
            nc.scalar.activation(a_new[:, m:m + 1, :], src, major,
                                 bias=bias, scale=IWS)
            t = tmp_pool.tile([128, CH], FP8, tag="btmp", bufs=2, name="btmp")
            nc.scalar.activation(t[:], src, minor, bias=bias, scale=IWS)
            nc.vector.copy_predicated(
                a_new[:, m:m + 1, :],
                bmask_t[:, mb * CH:(mb + 1) * CH], t[:])
        else:
            nc.scalar.activation(a_new[:, m:m + 1, :], src, _TILE_FUNC[m],
                                 bias=bias, scale=IWS)


def _build_nc():
    nc = bacc.Bacc("TRN2", target_bir_lowering=False, debug=False,
                   num_devices=N_CORES, dynamic_dma_scratch_size=2048)

    # all operands host-packed so each loads as one large contiguous DMA;
    # k-tile k of a weight lives at [:, k, :] so DoubleRow k-pairs are
    # adjacent in the middle dim
    xT = nc.dram_tensor("xT", [128, KI, BL], FP8, kind="ExternalInput").ap()
    wih = nc.dram_tensor("wih", [128, KI, NH], FP8, kind="ExternalInput").ap()
    whh = nc.dram_tensor("whh", [4 * 128, 4, NH], FP8,
                         kind="ExternalInput").ap()
    who = nc.dram_tensor("who", [128, KH, NO], FP8, kind="ExternalInput").ap()
    wio = nc.dram_tensor("wio", [128, KI, NO], FP8, kind="ExternalInput").ap()
    hbc = nc.dram_tensor("hbc", [128, KH], F32, kind="ExternalInput").ap()
    obc = nc.dram_tensor("obc", [128, KO], F32, kind="ExternalInput").ap()
    bmask = nc.dram_tensor("bmask", [128, 2 * CH], mybir.dt.uint8,
                           kind="ExternalInput").ap()
    outT = nc.dram_tensor("outT", [NO, BL], BF16, kind="ExternalOutput").ap()

    with tile.TileContext(nc) as tc:
        with tc.tile_pool(name="w", bufs=1) as wpool, \
             tc.tile_pool(name="act", bufs=1) as apool, \
             tc.tile_pool(name="ps", bufs=2, space="PSUM") as pspool, \
             tc.tile_pool(name="out", bufs=4) as opool:

            # ---- stage inputs across all three DMA trigger paths (SP +
            # ACT HWDGE queues, GpSimd software DGE), ordered first-needed-
            # first. The first x-proj PSUM group (m-tiles 0-3) only needs
            # wih cols 0:1024 of both k-pairs + the chunk-0 x columns, so
            # those small pieces lead the sync queue and the first matmul
            # can issue ~3us after queue start. ----
            wih_m = wpool.tile([128, KI, NH], FP8, tag="projA", name="wihm")
            x_m = wpool.tile([128, KI, BL], FP8, tag="x", name="xm")
            hbc_t = wpool.tile([128, KH], F32, tag="hbc")
            obc_t = wpool.tile([128, KO], F32, tag="obc")
            bmask_t = wpool.tile([128, 2 * CH], mybir.dt.uint8, tag="bmask")
            wio_m = wpool.tile([128, KI, NO], FP8, tag="wio", name="wiom")
            whh_m = [wpool.tile([128, 4, NH], FP8, tag=f"whhJ{J}",
                                name=f"whhJ{J}") for J in range(4)]
            # ---- PE warmup: ~3.5us of dummy matmuls on an uninitialized
            # tile (no DMA dependency) so the HAM clock-gate is at 8/8
            # before the first real matmul issues (~12us in, behind DMA) ----
            warm_t = wpool.tile([128, 2, CH], FP8, tag="warm", name="warm")
            nc.gpsimd.memset(warm_t[:], 0.0)
            ps_w = pspool.tile([128, 4 * CH], F32, tag="ps", name="psw")
            for _w in range(16):
                nc.tensor.matmul(
                    ps_w[:, (_w % 4) * CH:(_w % 4 + 1) * CH],
                    warm_t[:, :, 0:128], warm_t[:],
                    start=True, stop=True, perf_mode=DR,
                    skip_group_check=True)

            # the critical first 1.5MB (wih + chunk-0 x) splits across all
            # three trigger paths so the x-proj stream starts ~12us in;
            # gpsimd only ever carries contiguous-per-partition transfers
            # (strided software-DGE DMAs complete their semaphore
            # unreliably)
            nc.sync.dma_start(wih_m[:, 0:2, :], wih[:, 0:2, :])
            nc.gpsimd.dma_start(wih_m[:, 2:4, :], wih[:, 2:4, :])
            nc.scalar.dma_start(x_m[:, 0:2, 0:CH], xT[:, 0:2, 0:CH])
            nc.scalar.dma_start(x_m[:, 2:4, 0:CH], xT[:, 2:4, 0:CH])
            nc.scalar.dma_start(hbc_t[:], hbc[:])
            nc.scalar.dma_start(x_m[:, 0:2, CH:BL], xT[:, 0:2, CH:BL])
            nc.scalar.dma_start(x_m[:, 2:4, CH:BL], xT[:, 2:4, CH:BL])
            nc.scalar.dma_start(bmask_t[:], bmask[:])
            nc.gpsimd.dma_start(wio_m[:], wio[:])
            # whh J0-J2 land in m-column pieces, m-major, so the first hh
            # step's m-block b only waits on pieces 0..b and DMA pipelines
            # against the m-block progression; J3 rides the gpsimd software
            # DGE as one contiguous transfer (strided gpsimd DMAs complete
            # their semaphore unreliably)
            nc.gpsimd.dma_start(whh_m[3][:], whh[3 * 128:4 * 128])
            for mb in range(4):
                lo, hi = mb * 512, (mb + 1) * 512
                for J, eng in ((0, nc.sync), (1, nc.scalar), (2, nc.sync)):
                    eng.dma_start(whh_m[J][:, :, lo:hi],
                                  whh[J * 128:(J + 1) * 128, :, lo:hi])
            nc.scalar.dma_start(obc_t[:], obc[:])

            def whh_pair(kp):      # lhsT k-pair AP for hh k-tiles 2kp,2kp+1
                J, j = divmod(2 * kp, 4)
                return whh_m[J], j

            # ---- per-chunk x-projection P and first-step activations ----
            P = {}
            A = {}
            for c in range(NCH):
                P[c] = apool.tile([128, KH * CH], BF16, tag=f"P{c}",
                                  name=f"P{c}")
                a1 = apool.tile([128, KH, CH], FP8, tag="A", bufs=3,
                                name=f"A1c{c}")
                for blk in range(4):
                    ps = pspool.tile([128, 4 * CH], F32, tag="ps", name="psb")
                    for kp in range(KI // 2):
                        for mloc in range(4):
                            m = blk * 4 + mloc
                            nc.tensor.matmul(
                                ps[:, mloc * CH:(mloc + 1) * CH],
                                wih_m[:, 2 * kp:2 * kp + 2,
                                      m * 128:(m + 1) * 128],
                                x_m[:, 2 * kp:2 * kp + 2,
                                    c * CH:(c + 1) * CH],
                                start=(kp == 0), stop=(kp == KI // 2 - 1),
                                perf_mode=DR)
                    # P holds the raw WS-scaled x-projection (bias is applied
                    # inside the ACTs); a single copy frees the PSUM slot
                    nc.vector.tensor_copy(
                        P[c][:, blk * 4 * CH:(blk + 1) * 4 * CH], ps[:])
                    _emit_hidden_act(nc, P[c][:, blk * 4 * CH:(blk + 1) * 4 * CH],
                                     blk, a1, opool, bmask_t, hbc_t)
                A[c] = a1

            # ---- whh-independent output x-projection (fills the window
            # while the whh load is still in flight) ----
            outx = {}
            for c in range(NCH):
                outx[c] = apool.tile([128, KO * CH], BF16, tag=f"outx{c}",
                                     name=f"outx{c}")
                ps = pspool.tile([128, 4 * CH], F32, tag="ps", name="psb")
                for kp in range(KI // 2):
                    for mo in range(KO):
                        nc.tensor.matmul(
                            ps[:, mo * CH:(mo + 1) * CH],
                            wio_m[:, 2 * kp:2 * kp + 2,
                                  mo * 128:(mo + 1) * 128],
                            x_m[:, 2 * kp:2 * kp + 2, c * CH:(c + 1) * CH],
                            start=(kp == 0), stop=(kp == KI // 2 - 1),
                            perf_mode=DR)
                nc.vector.tensor_copy(outx[c][:], ps[:])

            # ---- recurrent steps 2..4 ----
            def hh_step(c, s):
                a_new = apool.tile([128, KH, CH], FP8, tag="A", bufs=3,
                                   name=f"A{s + 2}c{c}")
                for blk in range(4):
                    ps = pspool.tile([128, 4 * CH], F32, tag="ps", name="psb")
                    for kp in range(KH // 2):
                        wt, j = whh_pair(kp)
                        for mloc in range(4):
                            m = blk * 4 + mloc
                            nc.tensor.matmul(
                                ps[:, mloc * CH:(mloc + 1) * CH],
                                wt[:, j:j + 2, m * 128:(m + 1) * 128],
                                A[c][:, 2 * kp:2 * kp + 2, :],
                                start=(kp == 0), stop=(kp == KH // 2 - 1),
                                perf_mode=DR)
                    # pre = psum + P into an SBUF temp: a single PSUM read
                    # frees the bank; ACT then runs off SBUF
                    tmp = opool.tile([128, 4 * CH], F32, tag="pre", bufs=2,
                                     name="pre")
                    nc.vector.tensor_add(
                        tmp[:], ps[:], P[c][:, blk * 4 * CH:(blk + 1) * 4 * CH])
                    _emit_hidden_act(nc, tmp, blk, a_new, opool, bmask_t,
                                     hbc_t)
                A[c] = a_new

            for s in range(N_STEPS - 2):
                for c in range(NCH):
                    hh_step(c, s)
            hh_step(0, N_STEPS - 2)  # chunk 1's final step emitted after who

            # ---- output layer (who reuses the wih slot); chunk 0's
            # output overlaps chunk 1's final hh step ----
            who_m = wpool.tile([128, KH, NO], FP8, tag="projA", name="whom")
            nc.sync.dma_start(who_m[:], who[:])

            def out_chunk(c):
                for mo in range(KO):
                    pso = pspool.tile([128, CH], F32, tag="ps", name="pso")
                    oap = pso[:]
                    for kp in range(KH // 2):
                        nc.tensor.matmul(
                            oap,
                            who_m[:, 2 * kp:2 * kp + 2,
                                  mo * 128:(mo + 1) * 128],
                            A[c][:, 2 * kp:2 * kp + 2, :],
                            start=(kp == 0), stop=(kp == KH // 2 - 1),
                            perf_mode=DR)
                    to = opool.tile([128, CH], F32, tag="preo", bufs=2,
                                    name="preo")
                    nc.vector.tensor_add(
                        to[:], oap, outx[c][:, mo * CH:(mo + 1) * CH])
                    o = opool.tile([128, CH], BF16, tag="o", bufs=2, name="o")
                    nc.scalar.activation(o[:], to[:], AF.Sigmoid,
                                         bias=obc_t[:, mo:mo + 1], scale=IWS)
                    eng = nc.sync if mo % 2 == 0 else nc.scalar
                    eng.dma_start(
                        outT[mo * 128:(mo + 1) * 128, c * CH:(c + 1) * CH],
                        o[:])

            hh_step(1, N_STEPS - 2)
            out_chunk(0)
            out_chunk(1)

    nc.compile()
    return nc


_NC_CACHE = None


def _get_nc():
    global _NC_CACHE
    if _NC_CACHE is None:
        _NC_CACHE = _build_nc()
    return _NC_CACHE


def _make_bmask():
    m = np.zeros((128, 2 * CH), np.uint8)
    m[:_B1 - (_B1 // 128) * 128, 0:CH] = 1          # tile 5: parts < 43 tanh
    m[_B2 - (_B2 // 128) * 128:, CH:2 * CH] = 1     # tile 10: parts >= 86 relu
    return m


def _q8(a):
    return np.clip(a, -240.0, 240.0).astype(E4)


def _prep_in_maps(inputs):
    x = np.asarray(inputs["inputs"], np.float32)
    hr = np.asarray(inputs["hidden_responses"], np.float32)[PERM]
    hb = np.asarray(inputs["hidden_biases"], np.float32)[PERM]
    orr = np.asarray(inputs["output_responses"], np.float32)
    ob = np.asarray(inputs["output_biases"], np.float32)

    wih_s = WS * (hr[:, None] * np.asarray(inputs["input_to_hidden"], np.float32)[PERM]).T
    whh_s = WS * (hr[:, None] *
                  np.asarray(inputs["hidden_to_hidden"], np.float32)[PERM][:, PERM]).T
    who_s = WS * (orr[:, None] *
                  np.asarray(inputs["hidden_to_output"], np.float32)[:, PERM]).T
    wio_s = WS * (orr[:, None] * np.asarray(inputs["input_to_output"], np.float32)).T

    def pack(w, ktiles):     # (ktiles*128, C) -> (128, ktiles, C)
        c = w.shape[1]
        return np.ascontiguousarray(
            w.reshape(ktiles, 128, c).transpose(1, 0, 2))

    # who: [p, kk, c] = who_s[kk*128 + p, c]
    who_p = pack(who_s, KH)
    # whh: [J*128 + p, j, col] = whh_s[(4J+j)*128 + p, col]
    whh_p = np.ascontiguousarray(
        whh_s.reshape(4, 4, 128, NH).transpose(0, 2, 1, 3).reshape(
            4 * 128, 4, NH))

    shared = {
        "wih": _q8(pack(wih_s, KI)),
        "whh": _q8(whh_p),
        "who": _q8(who_p),
        "wio": _q8(pack(wio_s, KI)),
        "hbc": np.ascontiguousarray(hb.reshape(KH, 128).T),
        "obc": np.ascontiguousarray(ob.reshape(KO, 128).T),
        "bmask": _make_bmask(),
    }
    in_maps = []
    for c in range(N_CORES):
        m = dict(shared)
        m["xT"] = _q8(pack(np.ascontiguousarray(x[c * BL:(c + 1) * BL].T), KI))
        in_maps.append(m)
    return in_maps


def _run(inputs, trace=False, tmpdir=None):
    nc = _get_nc()
    in_maps = _prep_in_maps(inputs)
    res = run_bass_kernel_spmd(nc, in_maps, core_ids=list(range(N_CORES)),
                               trace=trace, tmpdir=tmpdir)
    out = np.empty((B, NO), np.float32)
    for c in range(N_CORES):
        out[c * BL:(c + 1) * BL] = res.results[c]["outT"].T.astype(np.float32)
    return out, res


def kernel(**inputs) -> np.ndarray:
    out, _ = _run(inputs, trace=False)
    return out


if __name__ == "__main__":
    rng = np.random.default_rng(0)
    ins = {
        "inputs": rng.standard_normal((B, NI), dtype=np.float32),
        "input_to_hidden": rng.standard_normal((NH, NI), dtype=np.float32) * 0.02,
        "hidden_to_hidden": rng.standard_normal((NH, NH), dtype=np.float32) * 0.02,
        "output_to_hidden": rng.standard_normal((NH, NO), dtype=np.float32) * 0.02,
        "input_to_output": rng.standard_normal((NO, NI), dtype=np.float32) * 0.02,
        "hidden_to_output": rng.standard_normal((NO, NH), dtype=np.float32) * 0.02,
        "output_to_output": rng.standard_normal((NO, NO), dtype=np.float32) * 0.02,
        "hidden_responses": rng.standard_normal(NH, dtype=np.float32) * 0.1 + 1.0,
        "hidden_biases": rng.standard_normal(NH, dtype=np.float32) * 0.1,
        "output_responses": rng.standard_normal(NO, dtype=np.float32) * 0.1 + 1.0,
        "output_biases": rng.standard_normal(NO, dtype=np.float32) * 0.1,
    }
    out = kernel(**ins)
    print("kernel output", out.shape, out.dtype, out[:2, :4])
